# revision 32
# baseline (speedup 1.0000x reference)
"""EnhancedSTGCN Trainium2 kernel (v4: (t,v)-inner layouts, no shadow copies).

Data-parallel over batch N=128 across 8 NeuronCores (16 samples/core),
processed as 8 pairs, two pairs in flight (couple) with per-block order
A(p0) B(p0) A(p1) B(p1) so each pair's EMA-scan tail overlaps the other
pair's PE work.

All activation tensors use (t, v)-inner free layouts:
  x  (per pair):   [4=(n,ci), (t, v)]      -- mm1 lhsT chunks are CONTIGUOUS
                                              126-col slices; res rhs is
                                              (t, v)-inner contiguous
  pd:              [c, (mh, n, t, v)]      -- tconv rhs (n, t-window, v) has
                                              stride-1 inner v runs
  us1:             [(n,c)=128, (t, v)]     -- block2 mm1 lhsT = one contiguous
                                              slice covering BOTH samples
  us2:             [c, (n, t, v)]
  us3:             [c, (mh, n, t, v)]

GraphConv keeps the transposed-chunk trick (mm1: lhsT = x/us chunk ->
(t,v) on PSUM partitions; mm2 vs blockdiag(A^T) applies adjacency and
transposes back).  v4 removes the v3 otv shadow copies entirely (the next
block reads the scanned us directly; GpSimd is unused), packs both samples
into one mm1 via blockdiag gw weights for blocks 1-2, and keeps every
ACT/DVE op stride-1 on its inner dim.

EMA is a masked tensor_tensor_scan; with (t, v) layout the scan runs per-v
as a strided 1-free-dim AP (mask [128, 256] is contiguous, 0 at t=0).

Block1's 9-tap tconv is tap-pair packed: pd held twice (rows 64:128
shifted by one tap = +V elements, one contiguous SBUF->SBUF DMA) -> K=128,
5 matmuls instead of 9.  BN1 scale folds into tconv weights host-side.

Matmul operands bf16 (fp32 PSUM).  All constants ship as ONE packed DRAM
tensor -> one DMA.
"""

import os
import sys

import numpy as np

sys.path.insert(0, "/opt/trn_rl_repo")

V = 18
T0 = 256
NS = 16  # samples per core
N_CORES = 8
ALPHA = 0.85
EPS = 1e-5
MM_MODE = "bf16"

# (ci, co, T_in, stride) per block
BLOCKS = [(2, 64, 256, 1), (64, 128, 256, 2), (128, 256, 128, 2)]

_NC_CACHE = {}


def _mm_np_dtype():
    if MM_MODE == "bf16":
        import ml_dtypes
        return ml_dtypes.bfloat16
    return np.float32


def _wlayout():
    """Packed weight layout: (name, rows, f32cols, kind) + offsets."""
    per = 2 if MM_MODE == "bf16" else 4
    def mmcols(width):
        return width * per // 4

    entries = [
        ("aexp", 128, mmcols(128), "mm"),
        ("smask", 128, 256, "f32"),
        ("smask128", 128, 256, "f32"),
        ("smask64", 128, 256, "f32"),
        ("gw1bd", 4, mmcols(128), "mm"),
        ("rw1bd", 4, mmcols(128), "mm"),
        ("twT1", 128, mmcols(5 * 64), "mm"),
        ("gw2bd", 128, mmcols(256), "mm"),
        ("rw2bd", 128, mmcols(256), "mm"),
        ("twT2", 128, mmcols(9 * 128), "mm"),
        ("gwT3", 128, mmcols(256), "mm"),
        ("rwT3", 128, mmcols(256), "mm"),
        ("twT3", 128, mmcols(9 * 2 * 256), "mm"),
    ]
    for l, (ci, co, T, stride) in enumerate(BLOCKS, 1):
        cop = min(co, 128)
        mhc = (co + 127) // 128
        entries.append((f"b1s_{l}", cop, mhc, "f32"))
        # b2c rows cover the us partition layout (128 for l=1: (n,c) dup)
        b2rows = 128 if l == 1 else cop
        entries.append((f"b2c_{l}", b2rows, mhc, "f32"))
        entries.append((f"b2c015_{l}", b2rows, mhc, "f32"))
    entries.append(("fcwT", 128, 20, "f32"))
    entries.append(("fcb", 1, 10, "f32"))
    off = 0
    layout = {}
    for name, rows, cols, kind in entries:
        layout[name] = (rows, off, cols, kind)
        off += cols
    return layout, off


def _build_nc(ns=NS):
    import concourse.bass as bass
    import concourse.tile as tile
    from concourse import bacc, mybir
    from contextlib import ExitStack

    F32 = mybir.dt.float32
    MDT = mybir.dt.bfloat16 if MM_MODE == "bf16" else F32
    AF = mybir.ActivationFunctionType
    OP = mybir.AluOpType

    layout, wtot = _wlayout()
    npairs = ns // 2

    nc = bacc.Bacc("TRN2", target_bir_lowering=False, debug=False)

    # x arrives bn-folded, per-sample layout [ci, (t, v)]
    x_d = nc.dram_tensor("x", [ns, 2, T0 * V], MDT, kind="ExternalInput")
    wpack_d = nc.dram_tensor("wpack", [128, wtot], F32, kind="ExternalInput")
    out_d = nc.dram_tensor("out", [ns, 10], F32, kind="ExternalOutput")

    with ExitStack() as ctx:
        tc = ctx.enter_context(tile.TileContext(nc))
        wp = ctx.enter_context(tc.tile_pool(name="wp", bufs=1))

        wtile = wp.tile([128, wtot], F32)
        nc.sync.dma_start(wtile[:], wpack_d[:])

        def wview(name):
            rows, off, cols, kind = layout[name]
            v = wtile[0:rows, off: off + cols]
            if kind == "mm" and MM_MODE == "bf16":
                v = v.bitcast(MDT)
            return v

        aexp = wview("aexp")
        smask = {1: wview("smask"), 2: wview("smask128"), 3: wview("smask64")}
        gw1bd = wview("gw1bd")
        rw1bd = wview("rw1bd")
        gw2bd = wview("gw2bd")
        rw2bd = wview("rw2bd")
        gwT3 = wview("gwT3")
        rwT3 = wview("rwT3")
        tw_s = {1: wview("twT1"), 2: wview("twT2"), 3: wview("twT3")}
        b1s_s = {l: wview(f"b1s_{l}") for l in (1, 2, 3)}
        b2c_s = {l: wview(f"b2c_{l}") for l in (1, 2, 3)}
        b2c015_s = {l: wview(f"b2c015_{l}") for l in (1, 2, 3)}
        fcw_s = wview("fcwT")
        fcb_s = wview("fcb")

        ones_t = wp.tile([1, ns], F32)
        nc.vector.memset(ones_t[:], 1.0)
        pooled = wp.tile([128, 2 * ns], F32)

        def chunk_list(total, step):
            full, rem = divmod(total, step)
            out = [(i * step, step) for i in range(full)]
            if rem:
                out.append((full * step, rem))
            return out

        with (
            tc.tile_pool(name="xp", bufs=4) as xp,
            tc.tile_pool(name="pdp", bufs=2) as pdp,
            tc.tile_pool(name="usp", bufs=3) as usp,
            tc.tile_pool(name="smp", bufs=4) as smp,
            tc.tile_pool(name="aps", bufs=2, space="PSUM") as aps,
            tc.tile_pool(name="bps", bufs=2, space="PSUM") as bps,
        ):
            rot = {"i": 0}
            # Pending EMA-scan thunks per pair.  Scans of pair A's block l
            # are EMITTED interleaved into pair B's chunk loops (1 per
            # chunk) so the DVE queue never forms a 20us scan wall that
            # blocks the next phase's DVE ops.  Tile's dependency tracking
            # keeps semantics (scans read us after B-acts, before the
            # pair's own next-block reads, which are emitted later).
            pend = {}

            def drain_scans(k=1):
                for pr in list(pend):
                    while pend[pr] and k > 0:
                        pend[pr].pop(0)()
                        k -= 1
                    if not pend[pr]:
                        del pend[pr]
                    if k <= 0:
                        break

            def a_phase(pr, l, usprev, xt, ci, co, T, stride):
                """GraphConv + BN1 + ReLU -> pd [c, (mh, n, t, v)]."""
                Tpad = T + 8
                mhc = (co + 127) // 128
                cop = min(co, 128)
                pdpp = 128 if l == 1 else cop
                pd = pdp.tile([pdpp, mhc * 2 * Tpad * V], MDT, tag="pd",
                              name=f"pd{l}_{pr}")
                pdm = pd.rearrange("p (s q) -> p s q", q=Tpad * V)
                nc.vector.memset(pdm[:, :, 0: 4 * V], 0.0)
                nc.vector.memset(pdm[:, :, (T + 4) * V: Tpad * V], 0.0)
                pd5 = pd.rearrange("p (mh n t v) -> p mh n t v",
                                   mh=mhc, n=2, v=V)

                def stage1(t0, tcn):
                    """mm1: x/us chunk (stationary) x gw -> ps1 [tv, n*co]."""
                    P = tcn * V
                    ps1 = aps.tile([126, 512], F32, tag="ps1", bufs=3,
                                   name=f"ps1_{l}_{pr}_{t0}")
                    if l == 1:
                        nc.tensor.matmul(ps1[0:P, 0:128],
                                         xt[0:4, t0 * V: t0 * V + P],
                                         gw1bd[0:4, :], start=True, stop=True)
                    elif l == 2:
                        nc.tensor.matmul(ps1[0:P, 0:256],
                                         usprev[0:128, t0 * V: t0 * V + P],
                                         gw2bd[:], start=True, stop=True)
                    else:
                        for n in range(2):
                            nc.tensor.matmul(
                                ps1[0:P, n * 256:(n + 1) * 256],
                                usprev[0:128,
                                       n * T * V + t0 * V:
                                       n * T * V + t0 * V + P],
                                gwT3[:], start=(n == 0), stop=(n == 1))
                    return ps1

                def stage2(t0, tcn, ps1):
                    """y1 copy, mm2 (adjacency+transpose), BN1+ReLU -> pd."""
                    P = tcn * V
                    y1 = smp.tile([126, 2 * co], MDT, tag="y1",
                                  name=f"y1_{l}_{pr}_{t0}")
                    # y1 is latency-critical (PE mm1->mm2 round trip): keep
                    # it on ACT, away from the scan-loaded DVE queue
                    nc.scalar.copy(y1[0:P, :], ps1[0:P, 0:2 * co])
                    for mh in range(mhc):
                        ps3 = aps.tile([cop, 512], F32, tag="ps3", bufs=3,
                                       name=f"ps3_{l}_{pr}_{t0}_{mh}")
                        for n in range(2):
                            nc.tensor.matmul(
                                ps3[:, n * P: n * P + P],
                                y1[0:P, n * co + mh * 128: n * co + mh * 128 + cop],
                                aexp[0:P, 0:P], start=(n == 0), stop=(n == 1))
                        # BN1+ReLU (s1 folded into tw): max(u + b1/s1, 0)
                        ps3v = ps3[:, 0:2 * P].rearrange(
                            "p (n t v) -> p n t v", n=2, v=V)
                        pdout = pd5[0:cop, mh, :, 4 + t0: 4 + t0 + tcn, :]
                        b1ap = b1s_s[l][:, mh: mh + 1]
                        if rot["i"] % 2 == 0:
                            nc.vector.tensor_scalar(pdout, ps3v, b1ap, 0.0,
                                                    OP.add, OP.max)
                        else:
                            nc.scalar.activation(pdout, ps3v, AF.Relu,
                                                 bias=b1ap)
                        rot["i"] += 1

                # 2-deep software pipeline: mm1(c+2) is emitted before
                # stage2(c), so mm2's y1 dependency has ~2 mm1s of PE
                # cover instead of stalling the in-order PE queue
                chunks = chunk_list(T, 7)
                pipe = []
                for ci_, (t0, tcn) in enumerate(chunks):
                    if ci_ % 2 == 0:
                        drain_scans(1)
                    pipe.append((t0, tcn, stage1(t0, tcn)))
                    if len(pipe) > 2:
                        stage2(*pipe.pop(0))
                while pipe:
                    stage2(*pipe.pop(0))
                if l == 1:
                    # tap-pair packing: rows 64:128 = rows 0:64 shifted one
                    # tap (= +V elements in the (t,v) flat layout)
                    nc.sync.dma_start(pdm[64:128, :, 0:(Tpad - 1) * V],
                                      pdm[0:64, :, V: Tpad * V])
                    nc.vector.memset(pdm[64:128, :, (Tpad - 1) * V: Tpad * V],
                                     0.0)
                return pd

            def bc_phase(pr, l, pd, usprev, xt, ci, co, T, stride):
                """tconv + residual -> us (+BN2+ReLU, EMA pre-scale); scans."""
                Tp = T // stride
                mhc = (co + 127) // 128
                khc = mhc
                cop = min(co, 128)
                # l==1 keeps n on partitions (rows (n,c)); l>=2 on columns
                uw = Tp * V if l == 1 else mhc * 2 * Tp * V
                us = usp.tile([128, uw], MDT, tag=f"us{l}",
                              name=f"us{l}_{pr}")
                pd5b = pd.rearrange("p (mh n t v) -> p mh n t v",
                                    mh=mhc, n=2, v=V)
                if l == 1:
                    xtv = xt.rearrange("p (t v) -> p t v", v=V)
                elif l == 2:
                    upv = usprev.rearrange("p (t v) -> p t v", v=V)
                else:
                    upv = usprev.rearrange("p (n t v) -> p n t v", n=2, v=V)
                if l == 2:
                    us4 = us.rearrange("p (n t v) -> p n t v", n=2, v=V)
                if l == 3:
                    us5 = us.rearrange("p (mh n t v) -> p mh n t v",
                                       mh=mhc, n=2, v=V)

                for mh in range(mhc):
                    for (t0, tcn) in chunk_list(Tp, 14):
                        # B-phase chunks have 1-4us of PE work each --
                        # plenty of cover for two scans
                        drain_scans(2)
                        NC1 = tcn * V
                        NC2 = 2 * NC1
                        pstc = bps.tile([cop, 512], F32, tag="pstc", bufs=2,
                                        name=f"pstc{l}_{pr}_{t0}_{mh}")
                        # residual
                        if l == 1:
                            for n in range(2):
                                rr = xtv[0:4, t0: t0 + tcn, :]
                                nc.tensor.matmul(
                                    pstc[:, n * NC1: n * NC1 + NC1],
                                    rw1bd[0:4, n * 64: n * 64 + 64],
                                    rr, start=(n == 0), stop=False)
                        elif l == 2:
                            # K=128 with blockdiag(rwT2) zero-padding: rows
                            # 64:128 (sample 1 data) hit zero weights for n=0
                            rr = upv[0:128,
                                     t0 * stride:
                                     (t0 + tcn - 1) * stride + 1: stride, :]
                            for n in range(2):
                                nc.tensor.matmul(
                                    pstc[:, n * NC1: n * NC1 + NC1],
                                    rw2bd[0:128, n * 128: n * 128 + cop],
                                    rr, start=(n == 0), stop=False)
                        else:
                            rr = upv[:, :, t0 * stride:
                                     (t0 + tcn - 1) * stride + 1: stride, :]
                            nc.tensor.matmul(pstc[:, 0:NC2],
                                             rwT3[:, mh * 128: mh * 128 + cop],
                                             rr, start=True, stop=False)
                        # tconv
                        if l == 1:
                            for j in range(5):
                                k = 2 * j
                                rhs = pd5b[:, 0, :, t0 + k: t0 + k + tcn, :]
                                nc.tensor.matmul(pstc[:, 0:NC2],
                                                 tw_s[1][:, j * 64: j * 64 + 64],
                                                 rhs,
                                                 start=False, stop=(j == 4))
                        else:
                            nmm = 9 * khc
                            i = 0
                            for k in range(9):
                                for kh in range(khc):
                                    rhs = pd5b[:, kh, :,
                                               stride * t0 + k:
                                               stride * t0 + k + (tcn - 1) * stride + 1:
                                               stride, :]
                                    woff = (k * khc + kh) * co + mh * 128
                                    nc.tensor.matmul(pstc[:, 0:NC2],
                                                     tw_s[l][:, woff: woff + cop],
                                                     rhs,
                                                     start=False, stop=(i == nmm - 1))
                                    i += 1
                        # BN2(+res bias)+ReLU -> us; t=0 col full scale (EMA
                        # s_0 = y_0), others pre-scaled by (1-ALPHA)
                        if l == 1:
                            usl1 = us.rearrange("p (t v) -> p t v", v=V)
                            for n in range(2):
                                src = pstc[0:64, n * NC1: n * NC1 + NC1] \
                                    .rearrange("p (t v) -> p t v", v=V)
                                dstv = usl1[n * 64: n * 64 + 64]
                                b2 = b2c_s[1][n * 64: n * 64 + 64, 0:1]
                                b215 = b2c015_s[1][n * 64: n * 64 + 64, 0:1]
                                if t0 == 0:
                                    nc.scalar.activation(
                                        dstv[:, 0:1, :], src[:, 0:1, :],
                                        AF.Relu, bias=b2)
                                    nc.scalar.activation(
                                        dstv[:, 1:tcn, :], src[:, 1:tcn, :],
                                        AF.Relu, bias=b215, scale=1.0 - ALPHA)
                                else:
                                    nc.scalar.activation(
                                        dstv[:, t0: t0 + tcn, :], src,
                                        AF.Relu, bias=b215, scale=1.0 - ALPHA)
                        else:
                            pstc4 = pstc[:, 0:NC2].rearrange(
                                "p (n t v) -> p n t v", n=2, v=V)
                            dst5 = us4 if l == 2 else us5[:, mh]
                            b2 = b2c_s[l][:, mh: mh + 1]
                            b215 = b2c015_s[l][:, mh: mh + 1]
                            if t0 == 0:
                                nc.scalar.activation(
                                    dst5[:, :, 0:1, :], pstc4[:, :, 0:1, :],
                                    AF.Relu, bias=b2)
                                nc.scalar.activation(
                                    dst5[:, :, 1:tcn, :], pstc4[:, :, 1:tcn, :],
                                    AF.Relu, bias=b215, scale=1.0 - ALPHA)
                            else:
                                nc.scalar.activation(
                                    dst5[:, :, t0: t0 + tcn, :], pstc4,
                                    AF.Relu, bias=b215, scale=1.0 - ALPHA)
                # EMA scans (DVE-only op).  One stride-V walk per v chains
                # through ALL (mh, n) segments: segs are contiguous Tp*V
                # blocks, so position v + V*i sweeps each seg's t-run in
                # order; the mask (0 at each seg's t=0) resets the
                # recurrence at seg boundaries.  Emission is deferred
                # (interleaved into the other pair's chunk loops).
                usv = us.rearrange("p (q v) -> p v q", v=V)

                def mk_scan(vv):
                    def emit():
                        seg = usv[:, vv, :]
                        nc.vector.tensor_tensor_scan(
                            seg, smask[l][:, 0:256], seg, 0.0,
                            OP.mult, OP.add)
                    return emit

                pend.setdefault(pr, []).extend(mk_scan(v) for v in range(V))
                return us

            Tp3 = BLOCKS[-1][2] // BLOCKS[-1][3]
            for cp in range(0, npairs, 3):
                couple = list(range(cp, min(cp + 3, npairs)))
                xts, uss = {}, {}
                for pr in couple:
                    xt = xp.tile([4, T0 * V], MDT, tag="x", name=f"x_{pr}")
                    nc.sync.dma_start(xt[0:2, :], x_d[pr * 2])
                    nc.sync.dma_start(xt[2:4, :], x_d[pr * 2 + 1])
                    xts[pr] = xt
                    uss[pr] = None
                for l, (ci, co, T, stride) in enumerate(BLOCKS, 1):
                    for pr in couple:
                        pd = a_phase(pr, l, uss[pr], xts[pr], ci, co, T, stride)
                        uss[pr] = bc_phase(pr, l, pd, uss[pr], xts[pr],
                                           ci, co, T, stride)
                # global mean pool (sum; 1/(64*18) folded into fc weights),
                # deferred into the next couple's drain stream (after that
                # pair's block-3 scans, which precede it in pend order)
                for pr in couple:
                    def mk_pool(pr, us3):
                        def emit():
                            for mh in range(2):
                                for n in range(2):
                                    base = (mh * 2 + n) * V * Tp3
                                    col = mh * ns + pr * 2 + n
                                    nc.vector.tensor_reduce(
                                        pooled[:, col: col + 1],
                                        us3[:, base: base + V * Tp3],
                                        axis=mybir.AxisListType.X, op=OP.add)
                        return emit
                    pend.setdefault(pr, []).append(mk_pool(pr, uss[pr]))
            # final flush before the FC head consumes `pooled`
            while pend:
                drain_scans(64)

        # ---- FC head ----
        with tc.tile_pool(name="fcps", bufs=1, space="PSUM") as fcps, \
             tc.tile_pool(name="fcout", bufs=1) as fcout:
            ps = fcps.tile([ns, 10], F32)
            nc.tensor.matmul(ps[:], pooled[:, 0:ns], fcw_s[:, 0:10],
                             start=True, stop=False)
            nc.tensor.matmul(ps[:], pooled[:, ns: 2 * ns], fcw_s[:, 10:20],
                             start=False, stop=False)
            nc.tensor.matmul(ps[:], ones_t[:], fcb_s[:], start=False, stop=True)
            osb = fcout.tile([ns, 10], F32)
            nc.scalar.copy(osb[:], ps[:])
            nc.sync.dma_start(out_d[:], osb[:])

    nc.compile()
    return nc


def _host_inputs(inputs, ns=NS):
    """Build the single packed weight tensor (replicated across cores)."""
    f32 = np.float32
    mdt = _mm_np_dtype()
    layout, wtot = _wlayout()
    wpack = np.zeros((128, wtot), f32)

    def put(name, arr):
        rows, off, cols, kind = layout[name]
        if kind == "mm":
            arr = np.ascontiguousarray(arr.astype(mdt))
            if MM_MODE == "bf16":
                assert arr.shape[-1] % 2 == 0
                wpack.view(np.uint32)[0:rows, off: off + cols] = arr.view(np.uint32)
                return
        arr = np.ascontiguousarray(arr.astype(f32))
        wpack[0:rows, off: off + cols] = arr

    A = np.asarray(inputs["A"], f32)
    aexp = np.zeros((128, 128), f32)
    for t in range(7):
        aexp[t * V:(t + 1) * V, t * V:(t + 1) * V] = A.T
    put("aexp", aexp)
    # EMA scan masks along the chained stride-V walk: 0 at each segment's
    # t=0 (recurrence reset), ALPHA elsewhere.  Segment length = Tp.
    for name, seglen in (("smask", 256), ("smask128", 128), ("smask64", 64)):
        mk = np.full((256,), f32(ALPHA))
        mk[::seglen] = 0.0
        put(name, np.broadcast_to(mk, (128, 256)))

    sc = {}
    for l, (ci, co, T, stride) in enumerate(BLOCKS, 1):
        g1 = np.asarray(inputs[f"l{l}_bn1g"], f32)
        g2 = np.asarray(inputs[f"l{l}_bn2g"], f32)
        sc[l] = (g1 / np.sqrt(f32(1.0) + f32(EPS)),
                 g2 / np.sqrt(f32(1.0) + f32(EPS)))

    gwT1 = np.asarray(inputs["l1_gw"], f32).T          # [2, 64]
    rwT1 = np.asarray(inputs["l1_rw"], f32)[:, :, 0, 0].T
    gw1bd = np.zeros((4, 128), f32)
    gw1bd[0:2, 0:64] = gwT1
    gw1bd[2:4, 64:128] = gwT1
    put("gw1bd", gw1bd)
    rw1bd = np.zeros((4, 128), f32)
    rw1bd[0:2, 0:64] = rwT1
    rw1bd[2:4, 64:128] = rwT1
    put("rw1bd", rw1bd)

    gwT2 = np.asarray(inputs["l2_gw"], f32).T          # [64, 128]
    rwT2 = np.asarray(inputs["l2_rw"], f32)[:, :, 0, 0].T
    gw2bd = np.zeros((128, 256), f32)
    gw2bd[0:64, 0:128] = gwT2
    gw2bd[64:128, 128:256] = gwT2
    put("gw2bd", gw2bd)
    rw2bd = np.zeros((128, 256), f32)
    rw2bd[0:64, 0:128] = rwT2
    rw2bd[64:128, 128:256] = rwT2
    put("rw2bd", rw2bd)

    put("gwT3", np.asarray(inputs["l3_gw"], f32).T)    # [128, 256]
    put("rwT3", np.asarray(inputs["l3_rw"], f32)[:, :, 0, 0].T)

    for l, (ci, co, T, stride) in enumerate(BLOCKS, 1):
        cop = min(co, 128)
        mhc = (co + 127) // 128
        khc = mhc
        s1, s2 = sc[l]
        tw = np.asarray(inputs[f"l{l}_tw"], f32)
        gb = np.asarray(inputs[f"l{l}_gb"], f32)
        bb1 = np.asarray(inputs[f"l{l}_bn1b"], f32)
        bb2 = np.asarray(inputs[f"l{l}_bn2b"], f32)
        tb = np.asarray(inputs[f"l{l}_tb"], f32)
        rb = np.asarray(inputs[f"l{l}_rb"], f32)
        b1v = s1 * gb + bb1
        b2c = s2 * tb + bb2 + rb
        b2c015 = f32(1.0 - ALPHA) * b2c
        # fold bn2 scale (out ch) AND bn1 scale (in ch) into tconv weights
        tws = tw * s2[:, None, None, None] * s1[None, :, None, None]
        if l == 1:
            twp = np.zeros((128, 5 * 64), f32)
            for j in range(5):
                twp[0:64, j * 64:(j + 1) * 64] = tws[:, :, 2 * j, 0].T
                if j < 4:
                    twp[64:128, j * 64:(j + 1) * 64] = tws[:, :, 2 * j + 1, 0].T
            put("twT1", twp)
        else:
            twp = np.zeros((cop, 9 * khc * co), f32)
            for k in range(9):
                for kh in range(khc):
                    blk = tws[:, kh * 128: kh * 128 + cop, k, 0].T
                    twp[:, (k * khc + kh) * co:(k * khc + kh + 1) * co] = blk
            put(f"twT{l}", twp)
        b1s = b1v / s1
        put(f"b1s_{l}", np.ascontiguousarray(b1s.reshape(mhc, cop).T))
        if l == 1:
            put("b2c_1", np.concatenate([b2c, b2c]).reshape(128, 1))
            put("b2c015_1", np.concatenate([b2c015, b2c015]).reshape(128, 1))
        else:
            put(f"b2c_{l}", np.ascontiguousarray(b2c.reshape(mhc, cop).T))
            put(f"b2c015_{l}",
                np.ascontiguousarray(b2c015.reshape(mhc, cop).T))
    fcw = np.asarray(inputs["fc_w"], f32)  # [10, 256]
    fcwT = fcw.T / f32(64 * V)  # fold mean pool
    put("fcwT", np.concatenate([fcwT[0:128, :], fcwT[128:256, :]], axis=1))
    put("fcb", np.asarray(inputs["fc_b"], f32).reshape(1, 10))
    return {"wpack": wpack}


def _host_x(inputs):
    """Apply data_bn (host fold), keep [n, ci, (t, v)] layout, cast."""
    f32 = np.float32
    x = np.asarray(inputs["x"], f32)  # (N, 2, 256, 18)
    s = (np.asarray(inputs["dbn_g"], f32)
         / np.sqrt(f32(1.0) + f32(EPS))).reshape(2, V)
    b = np.asarray(inputs["dbn_b"], f32).reshape(2, V)
    xb = x * s[None, :, None, :] + b[None, :, None, :]
    return np.ascontiguousarray(xb.reshape(x.shape[0], 2, T0 * V)).astype(_mm_np_dtype())


def kernel(**inputs) -> np.ndarray:
    from concourse.bass_utils import run_bass_kernel_spmd

    n_total = np.asarray(inputs["x"]).shape[0]
    ns = n_total // N_CORES
    key = ("nc", ns)
    if key not in _NC_CACHE:
        _NC_CACHE[key] = _build_nc(ns)
    nc = _NC_CACHE[key]

    shared = _host_inputs(inputs, ns)
    xb = _host_x(inputs)
    in_maps = []
    for c in range(N_CORES):
        m = dict(shared)
        m["x"] = np.ascontiguousarray(xb[c * ns:(c + 1) * ns])
        in_maps.append(m)

    res = run_bass_kernel_spmd(nc, in_maps, core_ids=list(range(N_CORES)))
    return np.concatenate([res.results[c]["out"] for c in range(N_CORES)], axis=0)


# revision 34
# speedup vs baseline: 1.0176x; 1.0176x over previous
"""EnhancedSTGCN Trainium2 kernel (v4: (t,v)-inner layouts, no shadow copies).

Data-parallel over batch N=128 across 8 NeuronCores (16 samples/core),
processed as 8 pairs, two pairs in flight (couple) with per-block order
A(p0) B(p0) A(p1) B(p1) so each pair's EMA-scan tail overlaps the other
pair's PE work.

All activation tensors use (t, v)-inner free layouts:
  x  (per pair):   [4=(n,ci), (t, v)]      -- mm1 lhsT chunks are CONTIGUOUS
                                              126-col slices; res rhs is
                                              (t, v)-inner contiguous
  pd:              [c, (mh, n, t, v)]      -- tconv rhs (n, t-window, v) has
                                              stride-1 inner v runs
  us1:             [(n,c)=128, (t, v)]     -- block2 mm1 lhsT = one contiguous
                                              slice covering BOTH samples
  us2:             [c, (n, t, v)]
  us3:             [c, (mh, n, t, v)]

GraphConv keeps the transposed-chunk trick (mm1: lhsT = x/us chunk ->
(t,v) on PSUM partitions; mm2 vs blockdiag(A^T) applies adjacency and
transposes back).  v4 removes the v3 otv shadow copies entirely (the next
block reads the scanned us directly; GpSimd is unused), packs both samples
into one mm1 via blockdiag gw weights for blocks 1-2, and keeps every
ACT/DVE op stride-1 on its inner dim.

EMA is a masked tensor_tensor_scan; with (t, v) layout the scan runs per-v
as a strided 1-free-dim AP (mask [128, 256] is contiguous, 0 at t=0).

Block1's 9-tap tconv is tap-pair packed: pd held twice (rows 64:128
shifted by one tap = +V elements, one contiguous SBUF->SBUF DMA) -> K=128,
5 matmuls instead of 9.  BN1 scale folds into tconv weights host-side.

Matmul operands bf16 (fp32 PSUM).  All constants ship as ONE packed DRAM
tensor -> one DMA.
"""

import os
import sys

import numpy as np

sys.path.insert(0, "/opt/trn_rl_repo")

V = 18
T0 = 256
NS = 16  # samples per core
N_CORES = 8
ALPHA = 0.85
EPS = 1e-5
MM_MODE = "bf16"

# (ci, co, T_in, stride) per block
BLOCKS = [(2, 64, 256, 1), (64, 128, 256, 2), (128, 256, 128, 2)]

_NC_CACHE = {}


def _mm_np_dtype():
    if MM_MODE == "bf16":
        import ml_dtypes
        return ml_dtypes.bfloat16
    return np.float32


def _wlayout():
    """Packed weight layout: (name, rows, f32cols, kind) + offsets."""
    per = 2 if MM_MODE == "bf16" else 4
    def mmcols(width):
        return width * per // 4

    entries = [
        ("aexp", 128, mmcols(128), "mm"),
        ("smask", 128, 256, "f32"),
        ("smask128", 128, 256, "f32"),
        ("smask64", 128, 256, "f32"),
        ("gw1bd", 4, mmcols(128), "mm"),
        ("rw1bd", 4, mmcols(128), "mm"),
        ("twT1", 128, mmcols(5 * 64), "mm"),
        ("gw2bd", 128, mmcols(256), "mm"),
        ("rw2bd", 128, mmcols(256), "mm"),
        ("twT2", 128, mmcols(9 * 128), "mm"),
        ("gwT3", 128, mmcols(256), "mm"),
        ("rwT3", 128, mmcols(256), "mm"),
        ("twT3", 128, mmcols(9 * 2 * 256), "mm"),
    ]
    for l, (ci, co, T, stride) in enumerate(BLOCKS, 1):
        cop = min(co, 128)
        mhc = (co + 127) // 128
        entries.append((f"b1s_{l}", cop, mhc, "f32"))
        # b2c rows cover the us partition layout (128 for l=1: (n,c) dup)
        b2rows = 128 if l == 1 else cop
        entries.append((f"b2c_{l}", b2rows, mhc, "f32"))
        entries.append((f"b2c015_{l}", b2rows, mhc, "f32"))
    entries.append(("fcwT", 128, 20, "f32"))
    entries.append(("fcb", 1, 10, "f32"))
    off = 0
    layout = {}
    for name, rows, cols, kind in entries:
        layout[name] = (rows, off, cols, kind)
        off += cols
    return layout, off


def _build_nc(ns=NS):
    import concourse.bass as bass
    import concourse.tile as tile
    from concourse import bacc, mybir
    from contextlib import ExitStack

    F32 = mybir.dt.float32
    MDT = mybir.dt.bfloat16 if MM_MODE == "bf16" else F32
    AF = mybir.ActivationFunctionType
    OP = mybir.AluOpType

    layout, wtot = _wlayout()
    npairs = ns // 2

    nc = bacc.Bacc("TRN2", target_bir_lowering=False, debug=False)

    # x arrives bn-folded, per-sample layout [ci, (t, v)]
    x_d = nc.dram_tensor("x", [ns, 2, T0 * V], MDT, kind="ExternalInput")
    wpack_d = nc.dram_tensor("wpack", [128, wtot], F32, kind="ExternalInput")
    out_d = nc.dram_tensor("out", [ns, 10], F32, kind="ExternalOutput")

    with ExitStack() as ctx:
        tc = ctx.enter_context(tile.TileContext(nc))
        wp = ctx.enter_context(tc.tile_pool(name="wp", bufs=1))

        wtile = wp.tile([128, wtot], F32)
        nc.sync.dma_start(wtile[:], wpack_d[:])

        def wview(name):
            rows, off, cols, kind = layout[name]
            v = wtile[0:rows, off: off + cols]
            if kind == "mm" and MM_MODE == "bf16":
                v = v.bitcast(MDT)
            return v

        aexp = wview("aexp")
        smask = {1: wview("smask"), 2: wview("smask128"), 3: wview("smask64")}
        gw1bd = wview("gw1bd")
        rw1bd = wview("rw1bd")
        gw2bd = wview("gw2bd")
        rw2bd = wview("rw2bd")
        gwT3 = wview("gwT3")
        rwT3 = wview("rwT3")
        tw_s = {1: wview("twT1"), 2: wview("twT2"), 3: wview("twT3")}
        b1s_s = {l: wview(f"b1s_{l}") for l in (1, 2, 3)}
        b2c_s = {l: wview(f"b2c_{l}") for l in (1, 2, 3)}
        b2c015_s = {l: wview(f"b2c015_{l}") for l in (1, 2, 3)}
        fcw_s = wview("fcwT")
        fcb_s = wview("fcb")

        ones_t = wp.tile([1, ns], F32)
        nc.vector.memset(ones_t[:], 1.0)
        pooled = wp.tile([128, 2 * ns], F32)

        def chunk_list(total, step):
            full, rem = divmod(total, step)
            out = [(i * step, step) for i in range(full)]
            if rem:
                out.append((full * step, rem))
            return out

        with (
            tc.tile_pool(name="xp", bufs=4) as xp,
            tc.tile_pool(name="pdp", bufs=2) as pdp,
            tc.tile_pool(name="usp", bufs=3) as usp,
            tc.tile_pool(name="smp", bufs=4) as smp,
            tc.tile_pool(name="aps", bufs=2, space="PSUM") as aps,
            tc.tile_pool(name="bps", bufs=2, space="PSUM") as bps,
        ):
            rot = {"i": 0}
            # Pending EMA-scan thunks per pair.  Scans of pair A's block l
            # are EMITTED interleaved into pair B's chunk loops (1 per
            # chunk) so the DVE queue never forms a 20us scan wall that
            # blocks the next phase's DVE ops.  Tile's dependency tracking
            # keeps semantics (scans read us after B-acts, before the
            # pair's own next-block reads, which are emitted later).
            pend = {}

            def drain_scans(k=1):
                for pr in list(pend):
                    while pend[pr] and k > 0:
                        pend[pr].pop(0)()
                        k -= 1
                    if not pend[pr]:
                        del pend[pr]
                    if k <= 0:
                        break

            def a_phase(pr, l, usprev, xt, ci, co, T, stride):
                """GraphConv + BN1 + ReLU -> pd [c, (mh, n, t, v)]."""
                Tpad = T + 8
                mhc = (co + 127) // 128
                cop = min(co, 128)
                pdpp = 128 if l == 1 else cop
                pd = pdp.tile([pdpp, mhc * 2 * Tpad * V], MDT, tag="pd",
                              name=f"pd{l}_{pr}")
                pdm = pd.rearrange("p (s q) -> p s q", q=Tpad * V)
                nc.vector.memset(pdm[:, :, 0: 4 * V], 0.0)
                nc.vector.memset(pdm[:, :, (T + 4) * V: Tpad * V], 0.0)
                pd5 = pd.rearrange("p (mh n t v) -> p mh n t v",
                                   mh=mhc, n=2, v=V)

                def stage1(t0, tcn):
                    """mm1: x/us chunk (stationary) x gw -> ps1 [tv, n*co]."""
                    P = tcn * V
                    ps1 = aps.tile([126, 512], F32, tag="ps1", bufs=3,
                                   name=f"ps1_{l}_{pr}_{t0}")
                    if l == 1:
                        nc.tensor.matmul(ps1[0:P, 0:128],
                                         xt[0:4, t0 * V: t0 * V + P],
                                         gw1bd[0:4, :], start=True, stop=True)
                    elif l == 2:
                        nc.tensor.matmul(ps1[0:P, 0:256],
                                         usprev[0:128, t0 * V: t0 * V + P],
                                         gw2bd[:], start=True, stop=True)
                    else:
                        for n in range(2):
                            nc.tensor.matmul(
                                ps1[0:P, n * 256:(n + 1) * 256],
                                usprev[0:128,
                                       n * T * V + t0 * V:
                                       n * T * V + t0 * V + P],
                                gwT3[:], start=(n == 0), stop=(n == 1))
                    return ps1

                def stage2(t0, tcn, ps1):
                    """y1 copy, mm2 (adjacency+transpose), BN1+ReLU -> pd."""
                    P = tcn * V
                    y1 = smp.tile([126, 2 * co], MDT, tag="y1",
                                  name=f"y1_{l}_{pr}_{t0}")
                    # 3:1 ACT:DVE -- DVE is loaded with the EMA scans
                    if rot["i"] % 4 == 0:
                        nc.vector.tensor_copy(y1[0:P, :], ps1[0:P, 0:2 * co])
                    else:
                        nc.scalar.copy(y1[0:P, :], ps1[0:P, 0:2 * co])
                    for mh in range(mhc):
                        ps3 = aps.tile([cop, 512], F32, tag="ps3", bufs=3,
                                       name=f"ps3_{l}_{pr}_{t0}_{mh}")
                        for n in range(2):
                            nc.tensor.matmul(
                                ps3[:, n * P: n * P + P],
                                y1[0:P, n * co + mh * 128: n * co + mh * 128 + cop],
                                aexp[0:P, 0:P], start=(n == 0), stop=(n == 1))
                        # BN1+ReLU (s1 folded into tw): max(u + b1/s1, 0)
                        ps3v = ps3[:, 0:2 * P].rearrange(
                            "p (n t v) -> p n t v", n=2, v=V)
                        pdout = pd5[0:cop, mh, :, 4 + t0: 4 + t0 + tcn, :]
                        b1ap = b1s_s[l][:, mh: mh + 1]
                        if rot["i"] % 4 == 2:
                            nc.vector.tensor_scalar(pdout, ps3v, b1ap, 0.0,
                                                    OP.add, OP.max)
                        else:
                            nc.scalar.activation(pdout, ps3v, AF.Relu,
                                                 bias=b1ap)
                        rot["i"] += 1

                # 2-deep software pipeline: mm1(c+2) is emitted before
                # stage2(c), so mm2's y1 dependency has ~2 mm1s of PE
                # cover instead of stalling the in-order PE queue
                chunks = chunk_list(T, 7)
                pipe = []
                for ci_, (t0, tcn) in enumerate(chunks):
                    if ci_ % 2 == 0:
                        drain_scans(1)
                    pipe.append((t0, tcn, stage1(t0, tcn)))
                    if len(pipe) > 2:
                        stage2(*pipe.pop(0))
                while pipe:
                    stage2(*pipe.pop(0))
                if l == 1:
                    # tap-pair packing: rows 64:128 = rows 0:64 shifted one
                    # tap (= +V elements in the (t,v) flat layout)
                    nc.sync.dma_start(pdm[64:128, :, 0:(Tpad - 1) * V],
                                      pdm[0:64, :, V: Tpad * V])
                    nc.vector.memset(pdm[64:128, :, (Tpad - 1) * V: Tpad * V],
                                     0.0)
                return pd

            def bc_phase(pr, l, pd, usprev, xt, ci, co, T, stride):
                """tconv + residual -> us (+BN2+ReLU, EMA pre-scale); scans."""
                Tp = T // stride
                mhc = (co + 127) // 128
                khc = mhc
                cop = min(co, 128)
                # l==1 keeps n on partitions (rows (n,c)); l>=2 on columns
                uw = Tp * V if l == 1 else mhc * 2 * Tp * V
                us = usp.tile([128, uw], MDT, tag=f"us{l}",
                              name=f"us{l}_{pr}")
                pd5b = pd.rearrange("p (mh n t v) -> p mh n t v",
                                    mh=mhc, n=2, v=V)
                if l == 1:
                    xtv = xt.rearrange("p (t v) -> p t v", v=V)
                elif l == 2:
                    upv = usprev.rearrange("p (t v) -> p t v", v=V)
                else:
                    upv = usprev.rearrange("p (n t v) -> p n t v", n=2, v=V)
                if l == 2:
                    us4 = us.rearrange("p (n t v) -> p n t v", n=2, v=V)
                if l == 3:
                    us5 = us.rearrange("p (mh n t v) -> p mh n t v",
                                       mh=mhc, n=2, v=V)

                for mh in range(mhc):
                    for (t0, tcn) in chunk_list(Tp, 14):
                        # B-phase chunks have 1-4us of PE work each --
                        # plenty of cover for two scans
                        drain_scans(2)
                        NC1 = tcn * V
                        NC2 = 2 * NC1
                        pstc = bps.tile([cop, 512], F32, tag="pstc", bufs=2,
                                        name=f"pstc{l}_{pr}_{t0}_{mh}")
                        # residual
                        if l == 1:
                            for n in range(2):
                                rr = xtv[0:4, t0: t0 + tcn, :]
                                nc.tensor.matmul(
                                    pstc[:, n * NC1: n * NC1 + NC1],
                                    rw1bd[0:4, n * 64: n * 64 + 64],
                                    rr, start=(n == 0), stop=False)
                        elif l == 2:
                            # K=128 with blockdiag(rwT2) zero-padding: rows
                            # 64:128 (sample 1 data) hit zero weights for n=0
                            rr = upv[0:128,
                                     t0 * stride:
                                     (t0 + tcn - 1) * stride + 1: stride, :]
                            for n in range(2):
                                nc.tensor.matmul(
                                    pstc[:, n * NC1: n * NC1 + NC1],
                                    rw2bd[0:128, n * 128: n * 128 + cop],
                                    rr, start=(n == 0), stop=False)
                        else:
                            rr = upv[:, :, t0 * stride:
                                     (t0 + tcn - 1) * stride + 1: stride, :]
                            nc.tensor.matmul(pstc[:, 0:NC2],
                                             rwT3[:, mh * 128: mh * 128 + cop],
                                             rr, start=True, stop=False)
                        # tconv
                        if l == 1:
                            for j in range(5):
                                k = 2 * j
                                rhs = pd5b[:, 0, :, t0 + k: t0 + k + tcn, :]
                                nc.tensor.matmul(pstc[:, 0:NC2],
                                                 tw_s[1][:, j * 64: j * 64 + 64],
                                                 rhs,
                                                 start=False, stop=(j == 4))
                        else:
                            nmm = 9 * khc
                            i = 0
                            for k in range(9):
                                for kh in range(khc):
                                    rhs = pd5b[:, kh, :,
                                               stride * t0 + k:
                                               stride * t0 + k + (tcn - 1) * stride + 1:
                                               stride, :]
                                    woff = (k * khc + kh) * co + mh * 128
                                    nc.tensor.matmul(pstc[:, 0:NC2],
                                                     tw_s[l][:, woff: woff + cop],
                                                     rhs,
                                                     start=False, stop=(i == nmm - 1))
                                    i += 1
                        # BN2(+res bias)+ReLU -> us; t=0 col full scale (EMA
                        # s_0 = y_0), others pre-scaled by (1-ALPHA)
                        if l == 1:
                            usl1 = us.rearrange("p (t v) -> p t v", v=V)
                            for n in range(2):
                                src = pstc[0:64, n * NC1: n * NC1 + NC1] \
                                    .rearrange("p (t v) -> p t v", v=V)
                                dstv = usl1[n * 64: n * 64 + 64]
                                b2 = b2c_s[1][n * 64: n * 64 + 64, 0:1]
                                b215 = b2c015_s[1][n * 64: n * 64 + 64, 0:1]
                                if t0 == 0:
                                    nc.scalar.activation(
                                        dstv[:, 0:1, :], src[:, 0:1, :],
                                        AF.Relu, bias=b2)
                                    nc.scalar.activation(
                                        dstv[:, 1:tcn, :], src[:, 1:tcn, :],
                                        AF.Relu, bias=b215, scale=1.0 - ALPHA)
                                else:
                                    nc.scalar.activation(
                                        dstv[:, t0: t0 + tcn, :], src,
                                        AF.Relu, bias=b215, scale=1.0 - ALPHA)
                        else:
                            pstc4 = pstc[:, 0:NC2].rearrange(
                                "p (n t v) -> p n t v", n=2, v=V)
                            dst5 = us4 if l == 2 else us5[:, mh]
                            b2 = b2c_s[l][:, mh: mh + 1]
                            b215 = b2c015_s[l][:, mh: mh + 1]
                            if t0 == 0:
                                nc.scalar.activation(
                                    dst5[:, :, 0:1, :], pstc4[:, :, 0:1, :],
                                    AF.Relu, bias=b2)
                                nc.scalar.activation(
                                    dst5[:, :, 1:tcn, :], pstc4[:, :, 1:tcn, :],
                                    AF.Relu, bias=b215, scale=1.0 - ALPHA)
                            else:
                                nc.scalar.activation(
                                    dst5[:, :, t0: t0 + tcn, :], pstc4,
                                    AF.Relu, bias=b215, scale=1.0 - ALPHA)
                # EMA scans (DVE-only op).  One stride-V walk per v chains
                # through ALL (mh, n) segments: segs are contiguous Tp*V
                # blocks, so position v + V*i sweeps each seg's t-run in
                # order; the mask (0 at each seg's t=0) resets the
                # recurrence at seg boundaries.  Emission is deferred
                # (interleaved into the other pair's chunk loops).
                usv = us.rearrange("p (q v) -> p v q", v=V)

                def mk_scan(vv):
                    def emit():
                        seg = usv[:, vv, :]
                        nc.vector.tensor_tensor_scan(
                            seg, smask[l][:, 0:256], seg, 0.0,
                            OP.mult, OP.add)
                    return emit

                pend.setdefault(pr, []).extend(mk_scan(v) for v in range(V))
                return us

            Tp3 = BLOCKS[-1][2] // BLOCKS[-1][3]
            for cp in range(0, npairs, 3):
                couple = list(range(cp, min(cp + 3, npairs)))
                xts, uss = {}, {}
                for pr in couple:
                    xt = xp.tile([4, T0 * V], MDT, tag="x", name=f"x_{pr}")
                    nc.sync.dma_start(xt[0:2, :], x_d[pr * 2])
                    nc.sync.dma_start(xt[2:4, :], x_d[pr * 2 + 1])
                    xts[pr] = xt
                    uss[pr] = None
                for l, (ci, co, T, stride) in enumerate(BLOCKS, 1):
                    for pr in couple:
                        pd = a_phase(pr, l, uss[pr], xts[pr], ci, co, T, stride)
                        uss[pr] = bc_phase(pr, l, pd, uss[pr], xts[pr],
                                           ci, co, T, stride)
                # global mean pool (sum; 1/(64*18) folded into fc weights),
                # deferred into the next couple's drain stream (after that
                # pair's block-3 scans, which precede it in pend order)
                for pr in couple:
                    def mk_pool(pr, us3):
                        def emit():
                            for mh in range(2):
                                for n in range(2):
                                    base = (mh * 2 + n) * V * Tp3
                                    col = mh * ns + pr * 2 + n
                                    nc.vector.tensor_reduce(
                                        pooled[:, col: col + 1],
                                        us3[:, base: base + V * Tp3],
                                        axis=mybir.AxisListType.X, op=OP.add)
                        return emit
                    pend.setdefault(pr, []).append(mk_pool(pr, uss[pr]))
            # final flush before the FC head consumes `pooled`
            while pend:
                drain_scans(64)

        # ---- FC head ----
        with tc.tile_pool(name="fcps", bufs=1, space="PSUM") as fcps, \
             tc.tile_pool(name="fcout", bufs=1) as fcout:
            ps = fcps.tile([ns, 10], F32)
            nc.tensor.matmul(ps[:], pooled[:, 0:ns], fcw_s[:, 0:10],
                             start=True, stop=False)
            nc.tensor.matmul(ps[:], pooled[:, ns: 2 * ns], fcw_s[:, 10:20],
                             start=False, stop=False)
            nc.tensor.matmul(ps[:], ones_t[:], fcb_s[:], start=False, stop=True)
            osb = fcout.tile([ns, 10], F32)
            nc.scalar.copy(osb[:], ps[:])
            nc.sync.dma_start(out_d[:], osb[:])

    nc.compile()
    return nc


def _host_inputs(inputs, ns=NS):
    """Build the single packed weight tensor (replicated across cores)."""
    f32 = np.float32
    mdt = _mm_np_dtype()
    layout, wtot = _wlayout()
    wpack = np.zeros((128, wtot), f32)

    def put(name, arr):
        rows, off, cols, kind = layout[name]
        if kind == "mm":
            arr = np.ascontiguousarray(arr.astype(mdt))
            if MM_MODE == "bf16":
                assert arr.shape[-1] % 2 == 0
                wpack.view(np.uint32)[0:rows, off: off + cols] = arr.view(np.uint32)
                return
        arr = np.ascontiguousarray(arr.astype(f32))
        wpack[0:rows, off: off + cols] = arr

    A = np.asarray(inputs["A"], f32)
    aexp = np.zeros((128, 128), f32)
    for t in range(7):
        aexp[t * V:(t + 1) * V, t * V:(t + 1) * V] = A.T
    put("aexp", aexp)
    # EMA scan masks along the chained stride-V walk: 0 at each segment's
    # t=0 (recurrence reset), ALPHA elsewhere.  Segment length = Tp.
    for name, seglen in (("smask", 256), ("smask128", 128), ("smask64", 64)):
        mk = np.full((256,), f32(ALPHA))
        mk[::seglen] = 0.0
        put(name, np.broadcast_to(mk, (128, 256)))

    sc = {}
    for l, (ci, co, T, stride) in enumerate(BLOCKS, 1):
        g1 = np.asarray(inputs[f"l{l}_bn1g"], f32)
        g2 = np.asarray(inputs[f"l{l}_bn2g"], f32)
        sc[l] = (g1 / np.sqrt(f32(1.0) + f32(EPS)),
                 g2 / np.sqrt(f32(1.0) + f32(EPS)))

    gwT1 = np.asarray(inputs["l1_gw"], f32).T          # [2, 64]
    rwT1 = np.asarray(inputs["l1_rw"], f32)[:, :, 0, 0].T
    gw1bd = np.zeros((4, 128), f32)
    gw1bd[0:2, 0:64] = gwT1
    gw1bd[2:4, 64:128] = gwT1
    put("gw1bd", gw1bd)
    rw1bd = np.zeros((4, 128), f32)
    rw1bd[0:2, 0:64] = rwT1
    rw1bd[2:4, 64:128] = rwT1
    put("rw1bd", rw1bd)

    gwT2 = np.asarray(inputs["l2_gw"], f32).T          # [64, 128]
    rwT2 = np.asarray(inputs["l2_rw"], f32)[:, :, 0, 0].T
    gw2bd = np.zeros((128, 256), f32)
    gw2bd[0:64, 0:128] = gwT2
    gw2bd[64:128, 128:256] = gwT2
    put("gw2bd", gw2bd)
    rw2bd = np.zeros((128, 256), f32)
    rw2bd[0:64, 0:128] = rwT2
    rw2bd[64:128, 128:256] = rwT2
    put("rw2bd", rw2bd)

    put("gwT3", np.asarray(inputs["l3_gw"], f32).T)    # [128, 256]
    put("rwT3", np.asarray(inputs["l3_rw"], f32)[:, :, 0, 0].T)

    for l, (ci, co, T, stride) in enumerate(BLOCKS, 1):
        cop = min(co, 128)
        mhc = (co + 127) // 128
        khc = mhc
        s1, s2 = sc[l]
        tw = np.asarray(inputs[f"l{l}_tw"], f32)
        gb = np.asarray(inputs[f"l{l}_gb"], f32)
        bb1 = np.asarray(inputs[f"l{l}_bn1b"], f32)
        bb2 = np.asarray(inputs[f"l{l}_bn2b"], f32)
        tb = np.asarray(inputs[f"l{l}_tb"], f32)
        rb = np.asarray(inputs[f"l{l}_rb"], f32)
        b1v = s1 * gb + bb1
        b2c = s2 * tb + bb2 + rb
        b2c015 = f32(1.0 - ALPHA) * b2c
        # fold bn2 scale (out ch) AND bn1 scale (in ch) into tconv weights
        tws = tw * s2[:, None, None, None] * s1[None, :, None, None]
        if l == 1:
            twp = np.zeros((128, 5 * 64), f32)
            for j in range(5):
                twp[0:64, j * 64:(j + 1) * 64] = tws[:, :, 2 * j, 0].T
                if j < 4:
                    twp[64:128, j * 64:(j + 1) * 64] = tws[:, :, 2 * j + 1, 0].T
            put("twT1", twp)
        else:
            twp = np.zeros((cop, 9 * khc * co), f32)
            for k in range(9):
                for kh in range(khc):
                    blk = tws[:, kh * 128: kh * 128 + cop, k, 0].T
                    twp[:, (k * khc + kh) * co:(k * khc + kh + 1) * co] = blk
            put(f"twT{l}", twp)
        b1s = b1v / s1
        put(f"b1s_{l}", np.ascontiguousarray(b1s.reshape(mhc, cop).T))
        if l == 1:
            put("b2c_1", np.concatenate([b2c, b2c]).reshape(128, 1))
            put("b2c015_1", np.concatenate([b2c015, b2c015]).reshape(128, 1))
        else:
            put(f"b2c_{l}", np.ascontiguousarray(b2c.reshape(mhc, cop).T))
            put(f"b2c015_{l}",
                np.ascontiguousarray(b2c015.reshape(mhc, cop).T))
    fcw = np.asarray(inputs["fc_w"], f32)  # [10, 256]
    fcwT = fcw.T / f32(64 * V)  # fold mean pool
    put("fcwT", np.concatenate([fcwT[0:128, :], fcwT[128:256, :]], axis=1))
    put("fcb", np.asarray(inputs["fc_b"], f32).reshape(1, 10))
    return {"wpack": wpack}


def _host_x(inputs):
    """Apply data_bn (host fold), keep [n, ci, (t, v)] layout, cast."""
    f32 = np.float32
    x = np.asarray(inputs["x"], f32)  # (N, 2, 256, 18)
    s = (np.asarray(inputs["dbn_g"], f32)
         / np.sqrt(f32(1.0) + f32(EPS))).reshape(2, V)
    b = np.asarray(inputs["dbn_b"], f32).reshape(2, V)
    xb = x * s[None, :, None, :] + b[None, :, None, :]
    return np.ascontiguousarray(xb.reshape(x.shape[0], 2, T0 * V)).astype(_mm_np_dtype())


def kernel(**inputs) -> np.ndarray:
    from concourse.bass_utils import run_bass_kernel_spmd

    n_total = np.asarray(inputs["x"]).shape[0]
    ns = n_total // N_CORES
    key = ("nc", ns)
    if key not in _NC_CACHE:
        _NC_CACHE[key] = _build_nc(ns)
    nc = _NC_CACHE[key]

    shared = _host_inputs(inputs, ns)
    xb = _host_x(inputs)
    in_maps = []
    for c in range(N_CORES):
        m = dict(shared)
        m["x"] = np.ascontiguousarray(xb[c * ns:(c + 1) * ns])
        in_maps.append(m)

    res = run_bass_kernel_spmd(nc, in_maps, core_ids=list(range(N_CORES)))
    return np.concatenate([res.results[c]["out"] for c in range(N_CORES)], axis=0)


# revision 36
# speedup vs baseline: 1.0901x; 1.0712x over previous
"""EnhancedSTGCN Trainium2 kernel (v4: (t,v)-inner layouts, no shadow copies).

Data-parallel over batch N=128 across 8 NeuronCores (16 samples/core),
processed as 8 pairs, two pairs in flight (couple) with per-block order
A(p0) B(p0) A(p1) B(p1) so each pair's EMA-scan tail overlaps the other
pair's PE work.

All activation tensors use (t, v)-inner free layouts:
  x  (per pair):   [4=(n,ci), (t, v)]      -- mm1 lhsT chunks are CONTIGUOUS
                                              126-col slices; res rhs is
                                              (t, v)-inner contiguous
  pd:              [c, (mh, n, t, v)]      -- tconv rhs (n, t-window, v) has
                                              stride-1 inner v runs
  us1:             [(n,c)=128, (t, v)]     -- block2 mm1 lhsT = one contiguous
                                              slice covering BOTH samples
  us2:             [c, (n, t, v)]
  us3:             [c, (mh, n, t, v)]

GraphConv keeps the transposed-chunk trick (mm1: lhsT = x/us chunk ->
(t,v) on PSUM partitions; mm2 vs blockdiag(A^T) applies adjacency and
transposes back).  v4 removes the v3 otv shadow copies entirely (the next
block reads the scanned us directly; GpSimd is unused), packs both samples
into one mm1 via blockdiag gw weights for blocks 1-2, and keeps every
ACT/DVE op stride-1 on its inner dim.

EMA is a masked tensor_tensor_scan; with (t, v) layout the scan runs per-v
as a strided 1-free-dim AP (mask [128, 256] is contiguous, 0 at t=0).

Block1's 9-tap tconv is tap-pair packed: pd held twice (rows 64:128
shifted by one tap = +V elements, one contiguous SBUF->SBUF DMA) -> K=128,
5 matmuls instead of 9.  BN1 scale folds into tconv weights host-side.

Matmul operands bf16 (fp32 PSUM).  All constants ship as ONE packed DRAM
tensor -> one DMA.
"""

import os
import sys

import numpy as np

sys.path.insert(0, "/opt/trn_rl_repo")

V = 18
T0 = 256
NS = 16  # samples per core
N_CORES = 8
ALPHA = 0.85
EPS = 1e-5
MM_MODE = "bf16"

# (ci, co, T_in, stride) per block
BLOCKS = [(2, 64, 256, 1), (64, 128, 256, 2), (128, 256, 128, 2)]

_NC_CACHE = {}


def _mm_np_dtype():
    if MM_MODE == "bf16":
        import ml_dtypes
        return ml_dtypes.bfloat16
    return np.float32


def _wlayout():
    """Packed weight layout: (name, rows, f32cols, kind) + offsets."""
    per = 2 if MM_MODE == "bf16" else 4
    def mmcols(width):
        return width * per // 4

    entries = [
        ("aexp", 128, mmcols(128), "mm"),
        ("smask", 128, 256, "f32"),
        ("smask128", 128, 256, "f32"),
        ("smask64", 128, 256, "f32"),
        ("gw1bd", 4, mmcols(128), "mm"),
        ("rw1bd", 4, mmcols(128), "mm"),
        ("twT1", 128, mmcols(5 * 64), "mm"),
        ("gw2bd", 128, mmcols(256), "mm"),
        ("rw2bd", 128, mmcols(256), "mm"),
        ("twT2", 128, mmcols(9 * 128), "mm"),
        ("gwT3", 128, mmcols(256), "mm"),
        ("rwT3", 128, mmcols(256), "mm"),
        ("twT3", 128, mmcols(9 * 2 * 256), "mm"),
    ]
    for l, (ci, co, T, stride) in enumerate(BLOCKS, 1):
        cop = min(co, 128)
        mhc = (co + 127) // 128
        entries.append((f"b1s_{l}", cop, mhc, "f32"))
        # b2c rows cover the us partition layout (128 for l=1: (n,c) dup)
        b2rows = 128 if l == 1 else cop
        entries.append((f"b2c_{l}", b2rows, mhc, "f32"))
        entries.append((f"b2c015_{l}", b2rows, mhc, "f32"))
    entries.append(("fcwT", 128, 20, "f32"))
    entries.append(("fcb", 1, 10, "f32"))
    off = 0
    layout = {}
    for name, rows, cols, kind in entries:
        layout[name] = (rows, off, cols, kind)
        off += cols
    return layout, off


def _build_nc(ns=NS):
    import concourse.bass as bass
    import concourse.tile as tile
    from concourse import bacc, mybir
    from contextlib import ExitStack

    F32 = mybir.dt.float32
    MDT = mybir.dt.bfloat16 if MM_MODE == "bf16" else F32
    AF = mybir.ActivationFunctionType
    OP = mybir.AluOpType

    layout, wtot = _wlayout()
    npairs = ns // 2

    nc = bacc.Bacc("TRN2", target_bir_lowering=False, debug=False)

    # x arrives bn-folded, per-sample layout [ci, (t, v)]
    x_d = nc.dram_tensor("x", [ns, 2, T0 * V], MDT, kind="ExternalInput")
    wpack_d = nc.dram_tensor("wpack", [128, wtot], F32, kind="ExternalInput")
    out_d = nc.dram_tensor("out", [ns, 10], F32, kind="ExternalOutput")

    with ExitStack() as ctx:
        tc = ctx.enter_context(tile.TileContext(nc))
        wp = ctx.enter_context(tc.tile_pool(name="wp", bufs=1))

        wtile = wp.tile([128, wtot], F32)
        nc.sync.dma_start(wtile[:], wpack_d[:])

        def wview(name):
            rows, off, cols, kind = layout[name]
            v = wtile[0:rows, off: off + cols]
            if kind == "mm" and MM_MODE == "bf16":
                v = v.bitcast(MDT)
            return v

        aexp = wview("aexp")
        smask = {1: wview("smask"), 2: wview("smask128"), 3: wview("smask64")}
        gw1bd = wview("gw1bd")
        rw1bd = wview("rw1bd")
        gw2bd = wview("gw2bd")
        rw2bd = wview("rw2bd")
        gwT3 = wview("gwT3")
        rwT3 = wview("rwT3")
        tw_s = {1: wview("twT1"), 2: wview("twT2"), 3: wview("twT3")}
        b1s_s = {l: wview(f"b1s_{l}") for l in (1, 2, 3)}
        b2c_s = {l: wview(f"b2c_{l}") for l in (1, 2, 3)}
        b2c015_s = {l: wview(f"b2c015_{l}") for l in (1, 2, 3)}
        fcw_s = wview("fcwT")
        fcb_s = wview("fcb")

        ones_t = wp.tile([1, ns], F32)
        nc.vector.memset(ones_t[:], 1.0)
        pooled = wp.tile([128, 2 * ns], F32)

        def chunk_list(total, step):
            full, rem = divmod(total, step)
            out = [(i * step, step) for i in range(full)]
            if rem:
                out.append((full * step, rem))
            return out

        with (
            tc.tile_pool(name="xp", bufs=4) as xp,
            tc.tile_pool(name="pdp", bufs=2) as pdp,
            tc.tile_pool(name="usp", bufs=3) as usp,
            tc.tile_pool(name="smp", bufs=4) as smp,
            tc.tile_pool(name="aps", bufs=2, space="PSUM") as aps,
            tc.tile_pool(name="bps", bufs=2, space="PSUM") as bps,
        ):
            rot = {"i": 0}
            # Pending EMA-scan thunks per pair.  Scans of pair A's block l
            # are EMITTED interleaved into pair B's chunk loops (1 per
            # chunk) so the DVE queue never forms a 20us scan wall that
            # blocks the next phase's DVE ops.  Tile's dependency tracking
            # keeps semantics (scans read us after B-acts, before the
            # pair's own next-block reads, which are emitted later).
            pend = {}

            def drain_scans(k=1):
                for pr in list(pend):
                    while pend[pr] and k > 0:
                        pend[pr].pop(0)()
                        k -= 1
                    if not pend[pr]:
                        del pend[pr]
                    if k <= 0:
                        break

            def a_phase(pr, l, usprev, xt, ci, co, T, stride):
                """GraphConv + BN1 + ReLU -> pd [c, (mh, n, t, v)]."""
                Tpad = T + 8
                mhc = (co + 127) // 128
                cop = min(co, 128)
                pdpp = 128 if l == 1 else cop
                pd = pdp.tile([pdpp, mhc * 2 * Tpad * V], MDT, tag="pd",
                              name=f"pd{l}_{pr}")
                pdm = pd.rearrange("p (s q) -> p s q", q=Tpad * V)
                nc.vector.memset(pdm[:, :, 0: 4 * V], 0.0)
                nc.vector.memset(pdm[:, :, (T + 4) * V: Tpad * V], 0.0)
                pd5 = pd.rearrange("p (mh n t v) -> p mh n t v",
                                   mh=mhc, n=2, v=V)

                # Chunks grouped so a group fills one 512-col PSUM bank:
                # B1 (2co=128) packs 4 chunks/bank, B2 (256) packs 2, B3
                # fills it with one.  ONE wide y1 copy per group amortizes
                # the ACT/DVE per-op overhead ~3x for B1.
                W = 2 * co if l < 3 else 512
                G = 512 // W

                def stage1(group):
                    """mm1s of a chunk group -> one ps1 bank [tv, j*W+...]"""
                    ps1 = aps.tile([126, 512], F32, tag="ps1", bufs=3,
                                   name=f"ps1_{l}_{pr}_{group[0][0]}")
                    nmm = len(group) * (2 if l == 3 else 1)
                    i = 0
                    for j, (t0, tcn) in enumerate(group):
                        P = tcn * V
                        if l == 1:
                            nc.tensor.matmul(
                                ps1[0:P, j * W: j * W + 128],
                                xt[0:4, t0 * V: t0 * V + P],
                                gw1bd[0:4, :],
                                start=(i == 0), stop=(i == nmm - 1))
                            i += 1
                        elif l == 2:
                            nc.tensor.matmul(
                                ps1[0:P, j * W: j * W + 256],
                                usprev[0:128, t0 * V: t0 * V + P],
                                gw2bd[:],
                                start=(i == 0), stop=(i == nmm - 1))
                            i += 1
                        else:
                            for n in range(2):
                                nc.tensor.matmul(
                                    ps1[0:P, n * 256:(n + 1) * 256],
                                    usprev[0:128,
                                           n * T * V + t0 * V:
                                           n * T * V + t0 * V + P],
                                    gwT3[:],
                                    start=(i == 0), stop=(i == nmm - 1))
                                i += 1
                    return ps1

                def stage2(group, ps1):
                    """y1 copy, mm2 (adjacency+transpose), BN1+ReLU -> pd."""
                    y1 = smp.tile([126, 512], MDT, tag="y1",
                                  name=f"y1_{l}_{pr}_{group[0][0]}")
                    wid = len(group) * W
                    mrows = max(tcn for _, tcn in group) * V
                    # 3:1 ACT:DVE -- DVE is loaded with the EMA scans
                    if rot["i"] % 4 == 0:
                        nc.vector.tensor_copy(y1[0:mrows, 0:wid],
                                              ps1[0:mrows, 0:wid])
                    else:
                        nc.scalar.copy(y1[0:mrows, 0:wid], ps1[0:mrows, 0:wid])
                    for j, (t0, tcn) in enumerate(group):
                        P = tcn * V
                        for mh in range(mhc):
                            ps3 = aps.tile([cop, 512], F32, tag="ps3", bufs=3,
                                           name=f"ps3_{l}_{pr}_{t0}_{mh}")
                            for n in range(2):
                                nc.tensor.matmul(
                                    ps3[:, n * P: n * P + P],
                                    y1[0:P, j * W + n * co + mh * 128:
                                       j * W + n * co + mh * 128 + cop],
                                    aexp[0:P, 0:P],
                                    start=(n == 0), stop=(n == 1))
                            # BN1+ReLU (s1 folded into tw): max(u+b1/s1, 0)
                            ps3v = ps3[:, 0:2 * P].rearrange(
                                "p (n t v) -> p n t v", n=2, v=V)
                            pdout = pd5[0:cop, mh, :, 4 + t0: 4 + t0 + tcn, :]
                            b1ap = b1s_s[l][:, mh: mh + 1]
                            if rot["i"] % 4 == 2:
                                nc.vector.tensor_scalar(pdout, ps3v, b1ap,
                                                        0.0, OP.add, OP.max)
                            else:
                                nc.scalar.activation(pdout, ps3v, AF.Relu,
                                                     bias=b1ap)
                            rot["i"] += 1

                # 2-deep software pipeline over groups: stage1(g+2) is
                # emitted before stage2(g), so mm2's y1 dependency has
                # whole groups of PE cover instead of stalling the
                # in-order PE queue
                chunks = chunk_list(T, 7)
                groups = [chunks[i: i + G] for i in range(0, len(chunks), G)]
                pipe = []
                for g in groups:
                    drain_scans(1)
                    pipe.append((g, stage1(g)))
                    if len(pipe) > 2:
                        stage2(*pipe.pop(0))
                while pipe:
                    stage2(*pipe.pop(0))
                if l == 1:
                    # tap-pair packing: rows 64:128 = rows 0:64 shifted one
                    # tap (= +V elements in the (t,v) flat layout)
                    nc.sync.dma_start(pdm[64:128, :, 0:(Tpad - 1) * V],
                                      pdm[0:64, :, V: Tpad * V])
                    nc.vector.memset(pdm[64:128, :, (Tpad - 1) * V: Tpad * V],
                                     0.0)
                return pd

            def bc_phase(pr, l, pd, usprev, xt, ci, co, T, stride):
                """tconv + residual -> us (+BN2+ReLU, EMA pre-scale); scans."""
                Tp = T // stride
                mhc = (co + 127) // 128
                khc = mhc
                cop = min(co, 128)
                # l==1 keeps n on partitions (rows (n,c)); l>=2 on columns
                uw = Tp * V if l == 1 else mhc * 2 * Tp * V
                us = usp.tile([128, uw], MDT, tag=f"us{l}",
                              name=f"us{l}_{pr}")
                pd5b = pd.rearrange("p (mh n t v) -> p mh n t v",
                                    mh=mhc, n=2, v=V)
                if l == 1:
                    xtv = xt.rearrange("p (t v) -> p t v", v=V)
                elif l == 2:
                    upv = usprev.rearrange("p (t v) -> p t v", v=V)
                else:
                    upv = usprev.rearrange("p (n t v) -> p n t v", n=2, v=V)
                if l == 2:
                    us4 = us.rearrange("p (n t v) -> p n t v", n=2, v=V)
                if l == 3:
                    us5 = us.rearrange("p (mh n t v) -> p mh n t v",
                                       mh=mhc, n=2, v=V)

                for mh in range(mhc):
                    for (t0, tcn) in chunk_list(Tp, 14):
                        # B-phase chunks have 1-4us of PE work each --
                        # plenty of cover for two scans
                        drain_scans(2)
                        NC1 = tcn * V
                        NC2 = 2 * NC1
                        pstc = bps.tile([cop, 512], F32, tag="pstc", bufs=2,
                                        name=f"pstc{l}_{pr}_{t0}_{mh}")
                        # residual
                        if l == 1:
                            for n in range(2):
                                rr = xtv[0:4, t0: t0 + tcn, :]
                                nc.tensor.matmul(
                                    pstc[:, n * NC1: n * NC1 + NC1],
                                    rw1bd[0:4, n * 64: n * 64 + 64],
                                    rr, start=(n == 0), stop=False)
                        elif l == 2:
                            # K=128 with blockdiag(rwT2) zero-padding: rows
                            # 64:128 (sample 1 data) hit zero weights for n=0
                            rr = upv[0:128,
                                     t0 * stride:
                                     (t0 + tcn - 1) * stride + 1: stride, :]
                            for n in range(2):
                                nc.tensor.matmul(
                                    pstc[:, n * NC1: n * NC1 + NC1],
                                    rw2bd[0:128, n * 128: n * 128 + cop],
                                    rr, start=(n == 0), stop=False)
                        else:
                            rr = upv[:, :, t0 * stride:
                                     (t0 + tcn - 1) * stride + 1: stride, :]
                            nc.tensor.matmul(pstc[:, 0:NC2],
                                             rwT3[:, mh * 128: mh * 128 + cop],
                                             rr, start=True, stop=False)
                        # tconv
                        if l == 1:
                            for j in range(5):
                                k = 2 * j
                                rhs = pd5b[:, 0, :, t0 + k: t0 + k + tcn, :]
                                nc.tensor.matmul(pstc[:, 0:NC2],
                                                 tw_s[1][:, j * 64: j * 64 + 64],
                                                 rhs,
                                                 start=False, stop=(j == 4))
                        else:
                            nmm = 9 * khc
                            i = 0
                            for k in range(9):
                                for kh in range(khc):
                                    rhs = pd5b[:, kh, :,
                                               stride * t0 + k:
                                               stride * t0 + k + (tcn - 1) * stride + 1:
                                               stride, :]
                                    woff = (k * khc + kh) * co + mh * 128
                                    nc.tensor.matmul(pstc[:, 0:NC2],
                                                     tw_s[l][:, woff: woff + cop],
                                                     rhs,
                                                     start=False, stop=(i == nmm - 1))
                                    i += 1
                        # BN2(+res bias)+ReLU -> us; t=0 col full scale (EMA
                        # s_0 = y_0), others pre-scaled by (1-ALPHA)
                        if l == 1:
                            usl1 = us.rearrange("p (t v) -> p t v", v=V)
                            for n in range(2):
                                src = pstc[0:64, n * NC1: n * NC1 + NC1] \
                                    .rearrange("p (t v) -> p t v", v=V)
                                dstv = usl1[n * 64: n * 64 + 64]
                                b2 = b2c_s[1][n * 64: n * 64 + 64, 0:1]
                                b215 = b2c015_s[1][n * 64: n * 64 + 64, 0:1]
                                if t0 == 0:
                                    nc.scalar.activation(
                                        dstv[:, 0:1, :], src[:, 0:1, :],
                                        AF.Relu, bias=b2)
                                    nc.scalar.activation(
                                        dstv[:, 1:tcn, :], src[:, 1:tcn, :],
                                        AF.Relu, bias=b215, scale=1.0 - ALPHA)
                                else:
                                    nc.scalar.activation(
                                        dstv[:, t0: t0 + tcn, :], src,
                                        AF.Relu, bias=b215, scale=1.0 - ALPHA)
                        else:
                            pstc4 = pstc[:, 0:NC2].rearrange(
                                "p (n t v) -> p n t v", n=2, v=V)
                            dst5 = us4 if l == 2 else us5[:, mh]
                            b2 = b2c_s[l][:, mh: mh + 1]
                            b215 = b2c015_s[l][:, mh: mh + 1]
                            if t0 == 0:
                                nc.scalar.activation(
                                    dst5[:, :, 0:1, :], pstc4[:, :, 0:1, :],
                                    AF.Relu, bias=b2)
                                nc.scalar.activation(
                                    dst5[:, :, 1:tcn, :], pstc4[:, :, 1:tcn, :],
                                    AF.Relu, bias=b215, scale=1.0 - ALPHA)
                            else:
                                nc.scalar.activation(
                                    dst5[:, :, t0: t0 + tcn, :], pstc4,
                                    AF.Relu, bias=b215, scale=1.0 - ALPHA)
                # EMA scans (DVE-only op).  One stride-V walk per v chains
                # through ALL (mh, n) segments: segs are contiguous Tp*V
                # blocks, so position v + V*i sweeps each seg's t-run in
                # order; the mask (0 at each seg's t=0) resets the
                # recurrence at seg boundaries.  Emission is deferred
                # (interleaved into the other pair's chunk loops).
                usv = us.rearrange("p (q v) -> p v q", v=V)

                def mk_scan(vv):
                    def emit():
                        seg = usv[:, vv, :]
                        nc.vector.tensor_tensor_scan(
                            seg, smask[l][:, 0:256], seg, 0.0,
                            OP.mult, OP.add)
                    return emit

                pend.setdefault(pr, []).extend(mk_scan(v) for v in range(V))
                return us

            Tp3 = BLOCKS[-1][2] // BLOCKS[-1][3]
            for cp in range(0, npairs, 3):
                couple = list(range(cp, min(cp + 3, npairs)))
                xts, uss = {}, {}
                for pr in couple:
                    xt = xp.tile([4, T0 * V], MDT, tag="x", name=f"x_{pr}")
                    nc.sync.dma_start(xt[0:2, :], x_d[pr * 2])
                    nc.sync.dma_start(xt[2:4, :], x_d[pr * 2 + 1])
                    xts[pr] = xt
                    uss[pr] = None
                for l, (ci, co, T, stride) in enumerate(BLOCKS, 1):
                    for pr in couple:
                        pd = a_phase(pr, l, uss[pr], xts[pr], ci, co, T, stride)
                        uss[pr] = bc_phase(pr, l, pd, uss[pr], xts[pr],
                                           ci, co, T, stride)
                # global mean pool (sum; 1/(64*18) folded into fc weights),
                # deferred into the next couple's drain stream (after that
                # pair's block-3 scans, which precede it in pend order)
                for pr in couple:
                    def mk_pool(pr, us3):
                        def emit():
                            for mh in range(2):
                                for n in range(2):
                                    base = (mh * 2 + n) * V * Tp3
                                    col = mh * ns + pr * 2 + n
                                    nc.vector.tensor_reduce(
                                        pooled[:, col: col + 1],
                                        us3[:, base: base + V * Tp3],
                                        axis=mybir.AxisListType.X, op=OP.add)
                        return emit
                    pend.setdefault(pr, []).append(mk_pool(pr, uss[pr]))
            # final flush before the FC head consumes `pooled`
            while pend:
                drain_scans(64)

        # ---- FC head ----
        with tc.tile_pool(name="fcps", bufs=1, space="PSUM") as fcps, \
             tc.tile_pool(name="fcout", bufs=1) as fcout:
            ps = fcps.tile([ns, 10], F32)
            nc.tensor.matmul(ps[:], pooled[:, 0:ns], fcw_s[:, 0:10],
                             start=True, stop=False)
            nc.tensor.matmul(ps[:], pooled[:, ns: 2 * ns], fcw_s[:, 10:20],
                             start=False, stop=False)
            nc.tensor.matmul(ps[:], ones_t[:], fcb_s[:], start=False, stop=True)
            osb = fcout.tile([ns, 10], F32)
            nc.scalar.copy(osb[:], ps[:])
            nc.sync.dma_start(out_d[:], osb[:])

    nc.compile()
    return nc


def _host_inputs(inputs, ns=NS):
    """Build the single packed weight tensor (replicated across cores)."""
    f32 = np.float32
    mdt = _mm_np_dtype()
    layout, wtot = _wlayout()
    wpack = np.zeros((128, wtot), f32)

    def put(name, arr):
        rows, off, cols, kind = layout[name]
        if kind == "mm":
            arr = np.ascontiguousarray(arr.astype(mdt))
            if MM_MODE == "bf16":
                assert arr.shape[-1] % 2 == 0
                wpack.view(np.uint32)[0:rows, off: off + cols] = arr.view(np.uint32)
                return
        arr = np.ascontiguousarray(arr.astype(f32))
        wpack[0:rows, off: off + cols] = arr

    A = np.asarray(inputs["A"], f32)
    aexp = np.zeros((128, 128), f32)
    for t in range(7):
        aexp[t * V:(t + 1) * V, t * V:(t + 1) * V] = A.T
    put("aexp", aexp)
    # EMA scan masks along the chained stride-V walk: 0 at each segment's
    # t=0 (recurrence reset), ALPHA elsewhere.  Segment length = Tp.
    for name, seglen in (("smask", 256), ("smask128", 128), ("smask64", 64)):
        mk = np.full((256,), f32(ALPHA))
        mk[::seglen] = 0.0
        put(name, np.broadcast_to(mk, (128, 256)))

    sc = {}
    for l, (ci, co, T, stride) in enumerate(BLOCKS, 1):
        g1 = np.asarray(inputs[f"l{l}_bn1g"], f32)
        g2 = np.asarray(inputs[f"l{l}_bn2g"], f32)
        sc[l] = (g1 / np.sqrt(f32(1.0) + f32(EPS)),
                 g2 / np.sqrt(f32(1.0) + f32(EPS)))

    gwT1 = np.asarray(inputs["l1_gw"], f32).T          # [2, 64]
    rwT1 = np.asarray(inputs["l1_rw"], f32)[:, :, 0, 0].T
    gw1bd = np.zeros((4, 128), f32)
    gw1bd[0:2, 0:64] = gwT1
    gw1bd[2:4, 64:128] = gwT1
    put("gw1bd", gw1bd)
    rw1bd = np.zeros((4, 128), f32)
    rw1bd[0:2, 0:64] = rwT1
    rw1bd[2:4, 64:128] = rwT1
    put("rw1bd", rw1bd)

    gwT2 = np.asarray(inputs["l2_gw"], f32).T          # [64, 128]
    rwT2 = np.asarray(inputs["l2_rw"], f32)[:, :, 0, 0].T
    gw2bd = np.zeros((128, 256), f32)
    gw2bd[0:64, 0:128] = gwT2
    gw2bd[64:128, 128:256] = gwT2
    put("gw2bd", gw2bd)
    rw2bd = np.zeros((128, 256), f32)
    rw2bd[0:64, 0:128] = rwT2
    rw2bd[64:128, 128:256] = rwT2
    put("rw2bd", rw2bd)

    put("gwT3", np.asarray(inputs["l3_gw"], f32).T)    # [128, 256]
    put("rwT3", np.asarray(inputs["l3_rw"], f32)[:, :, 0, 0].T)

    for l, (ci, co, T, stride) in enumerate(BLOCKS, 1):
        cop = min(co, 128)
        mhc = (co + 127) // 128
        khc = mhc
        s1, s2 = sc[l]
        tw = np.asarray(inputs[f"l{l}_tw"], f32)
        gb = np.asarray(inputs[f"l{l}_gb"], f32)
        bb1 = np.asarray(inputs[f"l{l}_bn1b"], f32)
        bb2 = np.asarray(inputs[f"l{l}_bn2b"], f32)
        tb = np.asarray(inputs[f"l{l}_tb"], f32)
        rb = np.asarray(inputs[f"l{l}_rb"], f32)
        b1v = s1 * gb + bb1
        b2c = s2 * tb + bb2 + rb
        b2c015 = f32(1.0 - ALPHA) * b2c
        # fold bn2 scale (out ch) AND bn1 scale (in ch) into tconv weights
        tws = tw * s2[:, None, None, None] * s1[None, :, None, None]
        if l == 1:
            twp = np.zeros((128, 5 * 64), f32)
            for j in range(5):
                twp[0:64, j * 64:(j + 1) * 64] = tws[:, :, 2 * j, 0].T
                if j < 4:
                    twp[64:128, j * 64:(j + 1) * 64] = tws[:, :, 2 * j + 1, 0].T
            put("twT1", twp)
        else:
            twp = np.zeros((cop, 9 * khc * co), f32)
            for k in range(9):
                for kh in range(khc):
                    blk = tws[:, kh * 128: kh * 128 + cop, k, 0].T
                    twp[:, (k * khc + kh) * co:(k * khc + kh + 1) * co] = blk
            put(f"twT{l}", twp)
        b1s = b1v / s1
        put(f"b1s_{l}", np.ascontiguousarray(b1s.reshape(mhc, cop).T))
        if l == 1:
            put("b2c_1", np.concatenate([b2c, b2c]).reshape(128, 1))
            put("b2c015_1", np.concatenate([b2c015, b2c015]).reshape(128, 1))
        else:
            put(f"b2c_{l}", np.ascontiguousarray(b2c.reshape(mhc, cop).T))
            put(f"b2c015_{l}",
                np.ascontiguousarray(b2c015.reshape(mhc, cop).T))
    fcw = np.asarray(inputs["fc_w"], f32)  # [10, 256]
    fcwT = fcw.T / f32(64 * V)  # fold mean pool
    put("fcwT", np.concatenate([fcwT[0:128, :], fcwT[128:256, :]], axis=1))
    put("fcb", np.asarray(inputs["fc_b"], f32).reshape(1, 10))
    return {"wpack": wpack}


def _host_x(inputs):
    """Apply data_bn (host fold), keep [n, ci, (t, v)] layout, cast."""
    f32 = np.float32
    x = np.asarray(inputs["x"], f32)  # (N, 2, 256, 18)
    s = (np.asarray(inputs["dbn_g"], f32)
         / np.sqrt(f32(1.0) + f32(EPS))).reshape(2, V)
    b = np.asarray(inputs["dbn_b"], f32).reshape(2, V)
    xb = x * s[None, :, None, :] + b[None, :, None, :]
    return np.ascontiguousarray(xb.reshape(x.shape[0], 2, T0 * V)).astype(_mm_np_dtype())


def kernel(**inputs) -> np.ndarray:
    from concourse.bass_utils import run_bass_kernel_spmd

    n_total = np.asarray(inputs["x"]).shape[0]
    ns = n_total // N_CORES
    key = ("nc", ns)
    if key not in _NC_CACHE:
        _NC_CACHE[key] = _build_nc(ns)
    nc = _NC_CACHE[key]

    shared = _host_inputs(inputs, ns)
    xb = _host_x(inputs)
    in_maps = []
    for c in range(N_CORES):
        m = dict(shared)
        m["x"] = np.ascontiguousarray(xb[c * ns:(c + 1) * ns])
        in_maps.append(m)

    res = run_bass_kernel_spmd(nc, in_maps, core_ids=list(range(N_CORES)))
    return np.concatenate([res.results[c]["out"] for c in range(N_CORES)], axis=0)


# revision 37
# speedup vs baseline: 1.1396x; 1.0454x over previous
"""EnhancedSTGCN Trainium2 kernel (v4: (t,v)-inner layouts, no shadow copies).

Data-parallel over batch N=128 across 8 NeuronCores (16 samples/core),
processed as 8 pairs, two pairs in flight (couple) with per-block order
A(p0) B(p0) A(p1) B(p1) so each pair's EMA-scan tail overlaps the other
pair's PE work.

All activation tensors use (t, v)-inner free layouts:
  x  (per pair):   [4=(n,ci), (t, v)]      -- mm1 lhsT chunks are CONTIGUOUS
                                              126-col slices; res rhs is
                                              (t, v)-inner contiguous
  pd:              [c, (mh, n, t, v)]      -- tconv rhs (n, t-window, v) has
                                              stride-1 inner v runs
  us1:             [(n,c)=128, (t, v)]     -- block2 mm1 lhsT = one contiguous
                                              slice covering BOTH samples
  us2:             [c, (n, t, v)]
  us3:             [c, (mh, n, t, v)]

GraphConv keeps the transposed-chunk trick (mm1: lhsT = x/us chunk ->
(t,v) on PSUM partitions; mm2 vs blockdiag(A^T) applies adjacency and
transposes back).  v4 removes the v3 otv shadow copies entirely (the next
block reads the scanned us directly; GpSimd is unused), packs both samples
into one mm1 via blockdiag gw weights for blocks 1-2, and keeps every
ACT/DVE op stride-1 on its inner dim.

EMA is a masked tensor_tensor_scan; with (t, v) layout the scan runs per-v
as a strided 1-free-dim AP (mask [128, 256] is contiguous, 0 at t=0).

Block1's 9-tap tconv is tap-pair packed: pd held twice (rows 64:128
shifted by one tap = +V elements, one contiguous SBUF->SBUF DMA) -> K=128,
5 matmuls instead of 9.  BN1 scale folds into tconv weights host-side.

Matmul operands bf16 (fp32 PSUM).  All constants ship as ONE packed DRAM
tensor -> one DMA.
"""

import os
import sys

import numpy as np

sys.path.insert(0, "/opt/trn_rl_repo")

V = 18
T0 = 256
NS = 16  # samples per core
N_CORES = 8
ALPHA = 0.85
EPS = 1e-5
MM_MODE = "bf16"

# (ci, co, T_in, stride) per block
BLOCKS = [(2, 64, 256, 1), (64, 128, 256, 2), (128, 256, 128, 2)]

_NC_CACHE = {}


def _mm_np_dtype():
    if MM_MODE == "bf16":
        import ml_dtypes
        return ml_dtypes.bfloat16
    return np.float32


def _wlayout():
    """Packed weight layout: (name, rows, f32cols, kind) + offsets."""
    per = 2 if MM_MODE == "bf16" else 4
    def mmcols(width):
        return width * per // 4

    entries = [
        ("aexp", 128, mmcols(128), "mm"),
        ("smask", 128, 256, "f32"),
        ("smask128", 128, 256, "f32"),
        ("smask64", 128, 256, "f32"),
        ("gw1bd", 4, mmcols(128), "mm"),
        ("rw1bd", 4, mmcols(128), "mm"),
        ("twT1", 128, mmcols(5 * 64), "mm"),
        ("gw2bd", 128, mmcols(256), "mm"),
        ("rw2bd", 128, mmcols(256), "mm"),
        ("twT2", 128, mmcols(9 * 128), "mm"),
        ("gwT3", 128, mmcols(256), "mm"),
        ("rwT3", 128, mmcols(256), "mm"),
        ("twT3", 128, mmcols(9 * 2 * 256), "mm"),
    ]
    for l, (ci, co, T, stride) in enumerate(BLOCKS, 1):
        cop = min(co, 128)
        mhc = (co + 127) // 128
        entries.append((f"b1s_{l}", cop, mhc, "f32"))
        # b2c rows cover the us partition layout (128 for l=1: (n,c) dup)
        b2rows = 128 if l == 1 else cop
        entries.append((f"b2c_{l}", b2rows, mhc, "f32"))
        entries.append((f"b2c015_{l}", b2rows, mhc, "f32"))
    entries.append(("fcwT", 128, 20, "f32"))
    entries.append(("fcb", 1, 10, "f32"))
    off = 0
    layout = {}
    for name, rows, cols, kind in entries:
        layout[name] = (rows, off, cols, kind)
        off += cols
    return layout, off


def _build_nc(ns=NS):
    import concourse.bass as bass
    import concourse.tile as tile
    from concourse import bacc, mybir
    from contextlib import ExitStack

    F32 = mybir.dt.float32
    MDT = mybir.dt.bfloat16 if MM_MODE == "bf16" else F32
    AF = mybir.ActivationFunctionType
    OP = mybir.AluOpType

    layout, wtot = _wlayout()
    npairs = ns // 2

    nc = bacc.Bacc("TRN2", target_bir_lowering=False, debug=False)

    # x arrives bn-folded, per-sample layout [ci, (t, v)]
    x_d = nc.dram_tensor("x", [ns, 2, T0 * V], MDT, kind="ExternalInput")
    wpack_d = nc.dram_tensor("wpack", [128, wtot], F32, kind="ExternalInput")
    out_d = nc.dram_tensor("out", [ns, 10], F32, kind="ExternalOutput")

    with ExitStack() as ctx:
        tc = ctx.enter_context(tile.TileContext(nc))
        wp = ctx.enter_context(tc.tile_pool(name="wp", bufs=1))

        wtile = wp.tile([128, wtot], F32)
        nc.sync.dma_start(wtile[:], wpack_d[:])

        def wview(name):
            rows, off, cols, kind = layout[name]
            v = wtile[0:rows, off: off + cols]
            if kind == "mm" and MM_MODE == "bf16":
                v = v.bitcast(MDT)
            return v

        aexp = wview("aexp")
        smask = {1: wview("smask"), 2: wview("smask128"), 3: wview("smask64")}
        gw1bd = wview("gw1bd")
        rw1bd = wview("rw1bd")
        gw2bd = wview("gw2bd")
        rw2bd = wview("rw2bd")
        gwT3 = wview("gwT3")
        rwT3 = wview("rwT3")
        tw_s = {1: wview("twT1"), 2: wview("twT2"), 3: wview("twT3")}
        b1s_s = {l: wview(f"b1s_{l}") for l in (1, 2, 3)}
        b2c_s = {l: wview(f"b2c_{l}") for l in (1, 2, 3)}
        b2c015_s = {l: wview(f"b2c015_{l}") for l in (1, 2, 3)}
        fcw_s = wview("fcwT")
        fcb_s = wview("fcb")

        ones_t = wp.tile([1, ns], F32)
        nc.vector.memset(ones_t[:], 1.0)
        pooled = wp.tile([128, 2 * ns], F32)

        def chunk_list(total, step):
            full, rem = divmod(total, step)
            out = [(i * step, step) for i in range(full)]
            if rem:
                out.append((full * step, rem))
            return out

        with (
            tc.tile_pool(name="xp", bufs=4) as xp,
            tc.tile_pool(name="pdp", bufs=2) as pdp,
            tc.tile_pool(name="usp", bufs=3) as usp,
            tc.tile_pool(name="smp", bufs=4) as smp,
            tc.tile_pool(name="aps", bufs=2, space="PSUM") as aps,
            tc.tile_pool(name="bps", bufs=2, space="PSUM") as bps,
        ):
            rot = {"i": 0}
            # Pending EMA-scan thunks per pair.  Scans of pair A's block l
            # are EMITTED interleaved into pair B's chunk loops (1 per
            # chunk) so the DVE queue never forms a 20us scan wall that
            # blocks the next phase's DVE ops.  Tile's dependency tracking
            # keeps semantics (scans read us after B-acts, before the
            # pair's own next-block reads, which are emitted later).
            pend = {}

            def drain_scans(k=1):
                for pr in list(pend):
                    while pend[pr] and k > 0:
                        pend[pr].pop(0)()
                        k -= 1
                    if not pend[pr]:
                        del pend[pr]
                    if k <= 0:
                        break

            def a_phase(pr, l, usprev, xt, ci, co, T, stride):
                """GraphConv + BN1 + ReLU -> pd [c, (mh, n, t, v)]."""
                Tpad = T + 8
                mhc = (co + 127) // 128
                cop = min(co, 128)
                pdpp = 128 if l == 1 else cop
                pd = pdp.tile([pdpp, mhc * 2 * Tpad * V], MDT, tag="pd",
                              name=f"pd{l}_{pr}")
                pdm = pd.rearrange("p (s q) -> p s q", q=Tpad * V)
                nc.vector.memset(pdm[:, :, 0: 4 * V], 0.0)
                nc.vector.memset(pdm[:, :, (T + 4) * V: Tpad * V], 0.0)
                pd5 = pd.rearrange("p (mh n t v) -> p mh n t v",
                                   mh=mhc, n=2, v=V)

                # Chunks grouped so a group fills one 512-col PSUM bank:
                # B1 (2co=128) packs 4 chunks/bank, B2 (256) packs 2, B3
                # fills it with one.  ONE wide y1 copy per group amortizes
                # the ACT/DVE per-op overhead ~3x for B1.
                W = 2 * co if l < 3 else 512
                G = 512 // W

                def stage1(group):
                    """mm1s of a chunk group -> one ps1 bank [tv, j*W+...]"""
                    ps1 = aps.tile([126, 512], F32, tag="ps1", bufs=3,
                                   name=f"ps1_{l}_{pr}_{group[0][0]}")
                    nmm = len(group) * (2 if l == 3 else 1)
                    i = 0
                    for j, (t0, tcn) in enumerate(group):
                        P = tcn * V
                        if l == 1:
                            nc.tensor.matmul(
                                ps1[0:P, j * W: j * W + 128],
                                xt[0:4, t0 * V: t0 * V + P],
                                gw1bd[0:4, :],
                                start=(i == 0), stop=(i == nmm - 1))
                            i += 1
                        elif l == 2:
                            nc.tensor.matmul(
                                ps1[0:P, j * W: j * W + 256],
                                usprev[0:128, t0 * V: t0 * V + P],
                                gw2bd[:],
                                start=(i == 0), stop=(i == nmm - 1))
                            i += 1
                        else:
                            for n in range(2):
                                nc.tensor.matmul(
                                    ps1[0:P, n * 256:(n + 1) * 256],
                                    usprev[0:128,
                                           n * T * V + t0 * V:
                                           n * T * V + t0 * V + P],
                                    gwT3[:],
                                    start=(i == 0), stop=(i == nmm - 1))
                                i += 1
                    return ps1

                def stage2(group, ps1):
                    """y1 copy, mm2 (adjacency+transpose), BN1+ReLU -> pd."""
                    y1 = smp.tile([126, 512], MDT, tag="y1",
                                  name=f"y1_{l}_{pr}_{group[0][0]}")
                    wid = len(group) * W
                    mrows = max(tcn for _, tcn in group) * V
                    # 3:1 ACT:DVE -- DVE is loaded with the EMA scans
                    if rot["i"] % 4 == 0:
                        nc.vector.tensor_copy(y1[0:mrows, 0:wid],
                                              ps1[0:mrows, 0:wid])
                    else:
                        nc.scalar.copy(y1[0:mrows, 0:wid], ps1[0:mrows, 0:wid])
                    # Pair adjacent FULL chunks (tcn=7): the two mm2s write
                    # one ps3 bank in (n, chunk) column order, so their t
                    # ranges merge and ONE activation covers both chunks.
                    pairs = []
                    k = 0
                    while k < len(group):
                        if (k + 1 < len(group) and group[k][1] == 7
                                and group[k + 1][1] == 7):
                            pairs.append(group[k: k + 2])
                            k += 2
                        else:
                            pairs.append(group[k: k + 1])
                            k += 1
                    for pgj, pg in enumerate(pairs):
                        np_ = len(pg)
                        j0 = group.index(pg[0])
                        P0 = pg[0][1] * V
                        tw_ = sum(tcn for _, tcn in pg)
                        for mh in range(mhc):
                            ps3 = aps.tile([cop, 512], F32, tag="ps3", bufs=3,
                                           name=f"ps3_{l}_{pr}_{pg[0][0]}_{mh}")
                            i = 0
                            for n in range(2):
                                for jj, (t0, tcn) in enumerate(pg):
                                    P = tcn * V
                                    nc.tensor.matmul(
                                        ps3[:, n * np_ * P0 + jj * P0:
                                            n * np_ * P0 + jj * P0 + P],
                                        y1[0:P, (j0 + jj) * W + n * co + mh * 128:
                                           (j0 + jj) * W + n * co + mh * 128 + cop],
                                        aexp[0:P, 0:P],
                                        start=(i == 0), stop=(i == 2 * np_ - 1))
                                    i += 1
                            # BN1+ReLU (s1 folded into tw): max(u+b1/s1, 0)
                            ps3v = ps3[:, 0:2 * tw_ * V].rearrange(
                                "p (n t v) -> p n t v", n=2, v=V)
                            t0 = pg[0][0]
                            pdout = pd5[0:cop, mh, :, 4 + t0: 4 + t0 + tw_, :]
                            b1ap = b1s_s[l][:, mh: mh + 1]
                            if rot["i"] % 4 == 2:
                                nc.vector.tensor_scalar(pdout, ps3v, b1ap,
                                                        0.0, OP.add, OP.max)
                            else:
                                nc.scalar.activation(pdout, ps3v, AF.Relu,
                                                     bias=b1ap)
                            rot["i"] += 1

                # 2-deep software pipeline over groups: stage1(g+2) is
                # emitted before stage2(g), so mm2's y1 dependency has
                # whole groups of PE cover instead of stalling the
                # in-order PE queue
                chunks = chunk_list(T, 7)
                groups = [chunks[i: i + G] for i in range(0, len(chunks), G)]
                pipe = []
                for g in groups:
                    drain_scans(1)
                    pipe.append((g, stage1(g)))
                    if len(pipe) > 2:
                        stage2(*pipe.pop(0))
                while pipe:
                    stage2(*pipe.pop(0))
                if l == 1:
                    # tap-pair packing: rows 64:128 = rows 0:64 shifted one
                    # tap (= +V elements in the (t,v) flat layout)
                    nc.sync.dma_start(pdm[64:128, :, 0:(Tpad - 1) * V],
                                      pdm[0:64, :, V: Tpad * V])
                    nc.vector.memset(pdm[64:128, :, (Tpad - 1) * V: Tpad * V],
                                     0.0)
                return pd

            def bc_phase(pr, l, pd, usprev, xt, ci, co, T, stride):
                """tconv + residual -> us (+BN2+ReLU, EMA pre-scale); scans."""
                Tp = T // stride
                mhc = (co + 127) // 128
                khc = mhc
                cop = min(co, 128)
                # l==1 keeps n on partitions (rows (n,c)); l>=2 on columns
                uw = Tp * V if l == 1 else mhc * 2 * Tp * V
                us = usp.tile([128, uw], MDT, tag=f"us{l}",
                              name=f"us{l}_{pr}")
                pd5b = pd.rearrange("p (mh n t v) -> p mh n t v",
                                    mh=mhc, n=2, v=V)
                if l == 1:
                    xtv = xt.rearrange("p (t v) -> p t v", v=V)
                elif l == 2:
                    upv = usprev.rearrange("p (t v) -> p t v", v=V)
                else:
                    upv = usprev.rearrange("p (n t v) -> p n t v", n=2, v=V)
                if l == 2:
                    us4 = us.rearrange("p (n t v) -> p n t v", n=2, v=V)
                if l == 3:
                    us5 = us.rearrange("p (mh n t v) -> p mh n t v",
                                       mh=mhc, n=2, v=V)

                for mh in range(mhc):
                    for (t0, tcn) in chunk_list(Tp, 14):
                        # B-phase chunks have 1-4us of PE work each --
                        # plenty of cover for two scans
                        drain_scans(2)
                        NC1 = tcn * V
                        NC2 = 2 * NC1
                        pstc = bps.tile([cop, 512], F32, tag="pstc", bufs=2,
                                        name=f"pstc{l}_{pr}_{t0}_{mh}")
                        # residual
                        if l == 1:
                            for n in range(2):
                                rr = xtv[0:4, t0: t0 + tcn, :]
                                nc.tensor.matmul(
                                    pstc[:, n * NC1: n * NC1 + NC1],
                                    rw1bd[0:4, n * 64: n * 64 + 64],
                                    rr, start=(n == 0), stop=False)
                        elif l == 2:
                            # K=128 with blockdiag(rwT2) zero-padding: rows
                            # 64:128 (sample 1 data) hit zero weights for n=0
                            rr = upv[0:128,
                                     t0 * stride:
                                     (t0 + tcn - 1) * stride + 1: stride, :]
                            for n in range(2):
                                nc.tensor.matmul(
                                    pstc[:, n * NC1: n * NC1 + NC1],
                                    rw2bd[0:128, n * 128: n * 128 + cop],
                                    rr, start=(n == 0), stop=False)
                        else:
                            rr = upv[:, :, t0 * stride:
                                     (t0 + tcn - 1) * stride + 1: stride, :]
                            nc.tensor.matmul(pstc[:, 0:NC2],
                                             rwT3[:, mh * 128: mh * 128 + cop],
                                             rr, start=True, stop=False)
                        # tconv
                        if l == 1:
                            for j in range(5):
                                k = 2 * j
                                rhs = pd5b[:, 0, :, t0 + k: t0 + k + tcn, :]
                                nc.tensor.matmul(pstc[:, 0:NC2],
                                                 tw_s[1][:, j * 64: j * 64 + 64],
                                                 rhs,
                                                 start=False, stop=(j == 4))
                        else:
                            nmm = 9 * khc
                            i = 0
                            for k in range(9):
                                for kh in range(khc):
                                    rhs = pd5b[:, kh, :,
                                               stride * t0 + k:
                                               stride * t0 + k + (tcn - 1) * stride + 1:
                                               stride, :]
                                    woff = (k * khc + kh) * co + mh * 128
                                    nc.tensor.matmul(pstc[:, 0:NC2],
                                                     tw_s[l][:, woff: woff + cop],
                                                     rhs,
                                                     start=False, stop=(i == nmm - 1))
                                    i += 1
                        # BN2(+res bias)+ReLU -> us; t=0 col full scale (EMA
                        # s_0 = y_0), others pre-scaled by (1-ALPHA)
                        if l == 1:
                            usl1 = us.rearrange("p (t v) -> p t v", v=V)
                            for n in range(2):
                                src = pstc[0:64, n * NC1: n * NC1 + NC1] \
                                    .rearrange("p (t v) -> p t v", v=V)
                                dstv = usl1[n * 64: n * 64 + 64]
                                b2 = b2c_s[1][n * 64: n * 64 + 64, 0:1]
                                b215 = b2c015_s[1][n * 64: n * 64 + 64, 0:1]
                                if t0 == 0:
                                    nc.scalar.activation(
                                        dstv[:, 0:1, :], src[:, 0:1, :],
                                        AF.Relu, bias=b2)
                                    nc.scalar.activation(
                                        dstv[:, 1:tcn, :], src[:, 1:tcn, :],
                                        AF.Relu, bias=b215, scale=1.0 - ALPHA)
                                else:
                                    nc.scalar.activation(
                                        dstv[:, t0: t0 + tcn, :], src,
                                        AF.Relu, bias=b215, scale=1.0 - ALPHA)
                        else:
                            pstc4 = pstc[:, 0:NC2].rearrange(
                                "p (n t v) -> p n t v", n=2, v=V)
                            dst5 = us4 if l == 2 else us5[:, mh]
                            b2 = b2c_s[l][:, mh: mh + 1]
                            b215 = b2c015_s[l][:, mh: mh + 1]
                            if t0 == 0:
                                nc.scalar.activation(
                                    dst5[:, :, 0:1, :], pstc4[:, :, 0:1, :],
                                    AF.Relu, bias=b2)
                                nc.scalar.activation(
                                    dst5[:, :, 1:tcn, :], pstc4[:, :, 1:tcn, :],
                                    AF.Relu, bias=b215, scale=1.0 - ALPHA)
                            else:
                                nc.scalar.activation(
                                    dst5[:, :, t0: t0 + tcn, :], pstc4,
                                    AF.Relu, bias=b215, scale=1.0 - ALPHA)
                # EMA scans (DVE-only op).  One stride-V walk per v chains
                # through ALL (mh, n) segments: segs are contiguous Tp*V
                # blocks, so position v + V*i sweeps each seg's t-run in
                # order; the mask (0 at each seg's t=0) resets the
                # recurrence at seg boundaries.  Emission is deferred
                # (interleaved into the other pair's chunk loops).
                usv = us.rearrange("p (q v) -> p v q", v=V)

                def mk_scan(vv):
                    def emit():
                        seg = usv[:, vv, :]
                        nc.vector.tensor_tensor_scan(
                            seg, smask[l][:, 0:256], seg, 0.0,
                            OP.mult, OP.add)
                    return emit

                pend.setdefault(pr, []).extend(mk_scan(v) for v in range(V))
                return us

            Tp3 = BLOCKS[-1][2] // BLOCKS[-1][3]
            for cp in range(0, npairs, 3):
                couple = list(range(cp, min(cp + 3, npairs)))
                xts, uss = {}, {}
                for pr in couple:
                    xt = xp.tile([4, T0 * V], MDT, tag="x", name=f"x_{pr}")
                    nc.sync.dma_start(xt[0:2, :], x_d[pr * 2])
                    nc.sync.dma_start(xt[2:4, :], x_d[pr * 2 + 1])
                    xts[pr] = xt
                    uss[pr] = None
                for l, (ci, co, T, stride) in enumerate(BLOCKS, 1):
                    for pr in couple:
                        pd = a_phase(pr, l, uss[pr], xts[pr], ci, co, T, stride)
                        uss[pr] = bc_phase(pr, l, pd, uss[pr], xts[pr],
                                           ci, co, T, stride)
                # global mean pool (sum; 1/(64*18) folded into fc weights),
                # deferred into the next couple's drain stream (after that
                # pair's block-3 scans, which precede it in pend order)
                for pr in couple:
                    def mk_pool(pr, us3):
                        def emit():
                            for mh in range(2):
                                for n in range(2):
                                    base = (mh * 2 + n) * V * Tp3
                                    col = mh * ns + pr * 2 + n
                                    nc.vector.tensor_reduce(
                                        pooled[:, col: col + 1],
                                        us3[:, base: base + V * Tp3],
                                        axis=mybir.AxisListType.X, op=OP.add)
                        return emit
                    pend.setdefault(pr, []).append(mk_pool(pr, uss[pr]))
            # final flush before the FC head consumes `pooled`
            while pend:
                drain_scans(64)

        # ---- FC head ----
        with tc.tile_pool(name="fcps", bufs=1, space="PSUM") as fcps, \
             tc.tile_pool(name="fcout", bufs=1) as fcout:
            ps = fcps.tile([ns, 10], F32)
            nc.tensor.matmul(ps[:], pooled[:, 0:ns], fcw_s[:, 0:10],
                             start=True, stop=False)
            nc.tensor.matmul(ps[:], pooled[:, ns: 2 * ns], fcw_s[:, 10:20],
                             start=False, stop=False)
            nc.tensor.matmul(ps[:], ones_t[:], fcb_s[:], start=False, stop=True)
            osb = fcout.tile([ns, 10], F32)
            nc.scalar.copy(osb[:], ps[:])
            nc.sync.dma_start(out_d[:], osb[:])

    nc.compile()
    return nc


def _host_inputs(inputs, ns=NS):
    """Build the single packed weight tensor (replicated across cores)."""
    f32 = np.float32
    mdt = _mm_np_dtype()
    layout, wtot = _wlayout()
    wpack = np.zeros((128, wtot), f32)

    def put(name, arr):
        rows, off, cols, kind = layout[name]
        if kind == "mm":
            arr = np.ascontiguousarray(arr.astype(mdt))
            if MM_MODE == "bf16":
                assert arr.shape[-1] % 2 == 0
                wpack.view(np.uint32)[0:rows, off: off + cols] = arr.view(np.uint32)
                return
        arr = np.ascontiguousarray(arr.astype(f32))
        wpack[0:rows, off: off + cols] = arr

    A = np.asarray(inputs["A"], f32)
    aexp = np.zeros((128, 128), f32)
    for t in range(7):
        aexp[t * V:(t + 1) * V, t * V:(t + 1) * V] = A.T
    put("aexp", aexp)
    # EMA scan masks along the chained stride-V walk: 0 at each segment's
    # t=0 (recurrence reset), ALPHA elsewhere.  Segment length = Tp.
    for name, seglen in (("smask", 256), ("smask128", 128), ("smask64", 64)):
        mk = np.full((256,), f32(ALPHA))
        mk[::seglen] = 0.0
        put(name, np.broadcast_to(mk, (128, 256)))

    sc = {}
    for l, (ci, co, T, stride) in enumerate(BLOCKS, 1):
        g1 = np.asarray(inputs[f"l{l}_bn1g"], f32)
        g2 = np.asarray(inputs[f"l{l}_bn2g"], f32)
        sc[l] = (g1 / np.sqrt(f32(1.0) + f32(EPS)),
                 g2 / np.sqrt(f32(1.0) + f32(EPS)))

    gwT1 = np.asarray(inputs["l1_gw"], f32).T          # [2, 64]
    rwT1 = np.asarray(inputs["l1_rw"], f32)[:, :, 0, 0].T
    gw1bd = np.zeros((4, 128), f32)
    gw1bd[0:2, 0:64] = gwT1
    gw1bd[2:4, 64:128] = gwT1
    put("gw1bd", gw1bd)
    rw1bd = np.zeros((4, 128), f32)
    rw1bd[0:2, 0:64] = rwT1
    rw1bd[2:4, 64:128] = rwT1
    put("rw1bd", rw1bd)

    gwT2 = np.asarray(inputs["l2_gw"], f32).T          # [64, 128]
    rwT2 = np.asarray(inputs["l2_rw"], f32)[:, :, 0, 0].T
    gw2bd = np.zeros((128, 256), f32)
    gw2bd[0:64, 0:128] = gwT2
    gw2bd[64:128, 128:256] = gwT2
    put("gw2bd", gw2bd)
    rw2bd = np.zeros((128, 256), f32)
    rw2bd[0:64, 0:128] = rwT2
    rw2bd[64:128, 128:256] = rwT2
    put("rw2bd", rw2bd)

    put("gwT3", np.asarray(inputs["l3_gw"], f32).T)    # [128, 256]
    put("rwT3", np.asarray(inputs["l3_rw"], f32)[:, :, 0, 0].T)

    for l, (ci, co, T, stride) in enumerate(BLOCKS, 1):
        cop = min(co, 128)
        mhc = (co + 127) // 128
        khc = mhc
        s1, s2 = sc[l]
        tw = np.asarray(inputs[f"l{l}_tw"], f32)
        gb = np.asarray(inputs[f"l{l}_gb"], f32)
        bb1 = np.asarray(inputs[f"l{l}_bn1b"], f32)
        bb2 = np.asarray(inputs[f"l{l}_bn2b"], f32)
        tb = np.asarray(inputs[f"l{l}_tb"], f32)
        rb = np.asarray(inputs[f"l{l}_rb"], f32)
        b1v = s1 * gb + bb1
        b2c = s2 * tb + bb2 + rb
        b2c015 = f32(1.0 - ALPHA) * b2c
        # fold bn2 scale (out ch) AND bn1 scale (in ch) into tconv weights
        tws = tw * s2[:, None, None, None] * s1[None, :, None, None]
        if l == 1:
            twp = np.zeros((128, 5 * 64), f32)
            for j in range(5):
                twp[0:64, j * 64:(j + 1) * 64] = tws[:, :, 2 * j, 0].T
                if j < 4:
                    twp[64:128, j * 64:(j + 1) * 64] = tws[:, :, 2 * j + 1, 0].T
            put("twT1", twp)
        else:
            twp = np.zeros((cop, 9 * khc * co), f32)
            for k in range(9):
                for kh in range(khc):
                    blk = tws[:, kh * 128: kh * 128 + cop, k, 0].T
                    twp[:, (k * khc + kh) * co:(k * khc + kh + 1) * co] = blk
            put(f"twT{l}", twp)
        b1s = b1v / s1
        put(f"b1s_{l}", np.ascontiguousarray(b1s.reshape(mhc, cop).T))
        if l == 1:
            put("b2c_1", np.concatenate([b2c, b2c]).reshape(128, 1))
            put("b2c015_1", np.concatenate([b2c015, b2c015]).reshape(128, 1))
        else:
            put(f"b2c_{l}", np.ascontiguousarray(b2c.reshape(mhc, cop).T))
            put(f"b2c015_{l}",
                np.ascontiguousarray(b2c015.reshape(mhc, cop).T))
    fcw = np.asarray(inputs["fc_w"], f32)  # [10, 256]
    fcwT = fcw.T / f32(64 * V)  # fold mean pool
    put("fcwT", np.concatenate([fcwT[0:128, :], fcwT[128:256, :]], axis=1))
    put("fcb", np.asarray(inputs["fc_b"], f32).reshape(1, 10))
    return {"wpack": wpack}


def _host_x(inputs):
    """Apply data_bn (host fold), keep [n, ci, (t, v)] layout, cast."""
    f32 = np.float32
    x = np.asarray(inputs["x"], f32)  # (N, 2, 256, 18)
    s = (np.asarray(inputs["dbn_g"], f32)
         / np.sqrt(f32(1.0) + f32(EPS))).reshape(2, V)
    b = np.asarray(inputs["dbn_b"], f32).reshape(2, V)
    xb = x * s[None, :, None, :] + b[None, :, None, :]
    return np.ascontiguousarray(xb.reshape(x.shape[0], 2, T0 * V)).astype(_mm_np_dtype())


def kernel(**inputs) -> np.ndarray:
    from concourse.bass_utils import run_bass_kernel_spmd

    n_total = np.asarray(inputs["x"]).shape[0]
    ns = n_total // N_CORES
    key = ("nc", ns)
    if key not in _NC_CACHE:
        _NC_CACHE[key] = _build_nc(ns)
    nc = _NC_CACHE[key]

    shared = _host_inputs(inputs, ns)
    xb = _host_x(inputs)
    in_maps = []
    for c in range(N_CORES):
        m = dict(shared)
        m["x"] = np.ascontiguousarray(xb[c * ns:(c + 1) * ns])
        in_maps.append(m)

    res = run_bass_kernel_spmd(nc, in_maps, core_ids=list(range(N_CORES)))
    return np.concatenate([res.results[c]["out"] for c in range(N_CORES)], axis=0)
